# revision 1
# baseline (speedup 1.0000x reference)
"""Trainium2 Bass kernel for nn_MiniAgentBlock (dense transformer block).

Sharding: DP=2 over batch x TP=4 within each batch (8 NeuronCores).
Core c: dp = c//4 (batch), tp = c%4 (4 q-heads / 1 kv-head, FF/4 slice).
All matmul phases run in transposed [feature, seq] layout with fp32r
matmuls (11-bit-mantissa inputs, fp32 accumulate). On-device AllReduce
after the attention output projection and ReduceScatter after the FFN
down projection, within each 4-core group. The residual x1 = x + attn is
folded into the ReduceScatter as 0.25*x1 per core, so the program is
identical on every core (pure SPMD, no core-dependent slicing).
"""
import sys
if "/opt/trn_rl_repo" not in sys.path:
    sys.path.insert(0, "/opt/trn_rl_repo")

import numpy as np
import concourse.bass as bass
import concourse.mybir as mybir
import concourse.tile as tile
from concourse import bacc
from concourse.bass_utils import run_bass_kernel_spmd

f32 = mybir.dt.float32
f32r = mybir.dt.float32r
AL = mybir.AluOpType
AF = mybir.ActivationFunctionType

B, S, H = 2, 2048, 2048
NH, NKV, HD = 16, 4, 128
FF = 5632
EPS = 1e-5
TPN = 4
QH = NH // TPN           # 4 q heads per core
FFS = FF // TPN          # 1408
FCT = FFS // 128         # 11 FF col tiles
SSL = S // TPN           # 512 output seq cols per core
NHT = H // 128           # 16 H tiles
NST = S // 128           # 16 seq tiles
NSB = S // 512           # 4 seq blocks
GROUPS = [[0, 1, 2, 3], [4, 5, 6, 7]]

# HD permutation: quadrant q: [evens 16q..16q+15 | odds 16q..16q+15]
PERM = np.zeros(HD, dtype=np.int64)
for _q in range(4):
    for _i in range(16):
        PERM[32 * _q + _i] = 2 * (16 * _q + _i)
        PERM[32 * _q + 16 + _i] = 2 * (16 * _q + _i) + 1
SHUF = [(i + 16) % 32 for i in range(32)]


def round_fp32r(a):
    u = np.ascontiguousarray(a, dtype=np.float32).view(np.uint32)
    low = u & np.uint32(0xFFF)
    keep = u >> np.uint32(12)
    round_up = (low > 0x800) | ((low == 0x800) & ((keep & 1) == 1))
    keep = keep + round_up.astype(np.uint32)
    return (keep << np.uint32(12)).view(np.float32)


def make_rope_tables(cos, sin, scale):
    C = np.zeros((HD, S), np.float32)
    S2 = np.zeros((HD, S), np.float32)
    for q in range(4):
        for i in range(16):
            pair = 16 * q + i
            C[32 * q + i] = cos[:, pair] * scale
            S2[32 * q + i] = -sin[:, pair] * scale
            C[32 * q + 16 + i] = cos[:, pair] * scale
            S2[32 * q + 16 + i] = sin[:, pair] * scale
    return C, S2


def _sb(x, sb):
    return x[:, sb * 512:(sb + 1) * 512]


def build(upto=10):
    L = upto
    nc = bacc.Bacc("TRN2", target_bir_lowering=False, debug=False,
                   num_devices=8)

    def din(name, shape, dt=f32r):
        return nc.dram_tensor(name, list(shape), dt, kind="ExternalInput")

    xT = din("xT", [H, S], f32)
    wq = din("wq", [H, TPN * HD])          # permuted cols, fp32r-rounded
    wk = din("wk", [H, HD])                # permuted cols
    wv = din("wv", [H, HD])
    wo = din("wo", [QH * HD, H])
    wg = din("wg", [H, FFS])
    wu = din("wu", [H, FFS])
    wd = din("wd", [FFS, H])
    cq = din("cq", [HD, S], f32)           # cos/sqrt(HD) in permuted layout
    s2q = din("s2q", [HD, S], f32)
    ck = din("ck", [HD, S], f32)
    s2k = din("s2k", [HD, S], f32)
    wn1 = din("wn1", [128, NHT], f32)      # w_norm1[ht*128+p] at [p, ht]
    wn2 = din("wn2", [128, NHT], f32)
    tri = din("tri", [128, 128])           # f32r 0/1, tri[k,i] = (i >= k)
    ones = din("ones", [128, 1])           # f32r ones
    epsb = din("epsb", [128, 1], f32)      # EPS bias tile
    ident = din("ident", [128, 128], f32)  # f32 identity
    outsl = nc.dram_tensor("outsl", [H, SSL], f32, kind="ExternalOutput")

    with tile.TileContext(nc) as tc:
        with tc.tile_pool(name="pconst", bufs=1) as pconst, \
             tc.tile_pool(name="pdram", bufs=1, space="DRAM") as pdram:
            ones_t = pconst.tile([128, 1], f32r)
            tri_t = pconst.tile([128, 128], f32r)
            id_t = pconst.tile([128, 128], f32)
            wn1_t = pconst.tile([128, NHT], f32)
            wn2_t = pconst.tile([128, NHT], f32)
            eps_t = pconst.tile([128, 1], f32)
            nc.sync.dma_start(ones_t[:], ones[:])
            nc.sync.dma_start(tri_t[:], tri[:])
            nc.sync.dma_start(id_t[:], ident[:])
            nc.sync.dma_start(wn1_t[:], wn1[:])
            nc.sync.dma_start(wn2_t[:], wn2[:])
            nc.sync.dma_start(eps_t[:], epsb[:])

            outd = pdram.tile([QH, 128, S], f32r)
            ar_in = [pdram.tile([H, 512], f32, name=f"ar_in{i}")
                     for i in range(NSB)]
            ar_out = [pdram.tile([H, 512], f32, name=f"ar_out{i}")
                      for i in range(NSB)]
            mTd = pdram.tile([FCT, 128, S], f32r)
            rs_in = pdram.tile([2, NSB, 1024, 512], f32)  # [hh, sb, r, c]
            rs_out = pdram.tile([H, 512], f32)

            with tc.tile_pool(name="phT", bufs=1) as phT:
                hT = phT.tile([128, NHT, S], f32r)

                # ---------- Phase A: rmsnorm1 -> hT ----------
                with tc.tile_pool(name="pA", bufs=1) as pA, \
                     tc.tile_pool(name="pAs", bufs=2) as pAs, \
                     tc.tile_pool(name="pAp", bufs=2, space="PSUM") as pAp:
                    for sb in range(NSB if L >= 1 else 0):
                        xsb = pA.tile([128, NHT, 512], f32, tag="xsb")
                        ss_ps = pAp.tile([1, 512], f32, tag="ss")
                        for ht in range(NHT):
                            nc.sync.dma_start(
                                xsb[:, ht, :],
                                _sb(xT[ht * 128:(ht + 1) * 128, :], sb))
                            sq = pAs.tile([128, 512], f32r, tag="sq")
                            nc.scalar.activation(sq[:], xsb[:, ht, :],
                                                 AF.Square)
                            nc.tensor.matmul(ss_ps[:], ones_t[:], sq[:],
                                             start=(ht == 0),
                                             stop=(ht == NHT - 1))
                        sd = pAs.tile([1, 512], f32, tag="sd")
                        nc.scalar.activation(sd[:], ss_ps[:], AF.Sqrt,
                                             bias=eps_t[0:1, :],
                                             scale=1.0 / H)
                        rr = pAs.tile([1, 512], f32, tag="rr")
                        nc.vector.reciprocal(rr[:], sd[:])
                        rb = pAs.tile([128, 512], f32, tag="rb")
                        nc.gpsimd.partition_broadcast(rb[:], rr[:])
                        for ht in range(NHT):
                            nc.vector.scalar_tensor_tensor(
                                out=_sb(hT[:, ht, :], sb),
                                in0=xsb[:, ht, :],
                                scalar=wn1_t[:, ht:ht + 1],
                                in1=rb[:], op0=AL.mult, op1=AL.mult)

                # ---------- Phase B: K/V projections + K rope ----------
                with tc.tile_pool(name="pkv", bufs=1) as pkv:
                    kT = pkv.tile([128, S], f32r)
                    v_nat = pkv.tile([128, NST, HD], f32r)

                    with tc.tile_pool(name="pB", bufs=1) as pB, \
                         tc.tile_pool(name="pBw", bufs=1) as pBw, \
                         tc.tile_pool(name="pBp", bufs=2,
                                      space="PSUM") as pBp:
                        wkt = pBw.tile([128, NHT, 128], f32r, tag="wB")
                        if L >= 2:
                            nc.sync.dma_start(
                                wkt[:],
                                wk.rearrange("(o p) n -> p o n", p=128))
                        for sb in range(NSB if L >= 2 else 0):
                            ps = pBp.tile([128, 512], f32, tag="proj")
                            for ht in range(NHT):
                                nc.tensor.matmul(
                                    ps[:], wkt[:, ht, :],
                                    _sb(hT[:, ht, :], sb),
                                    start=(ht == 0), stop=(ht == NHT - 1))
                            ct_t = pB.tile([128, 512], f32, tag="ropeC", bufs=1)
                            st_t = pB.tile([128, 512], f32, tag="ropeS", bufs=1)
                            nc.sync.dma_start(ct_t[:], _sb(ck, sb))
                            nc.sync.dma_start(st_t[:], _sb(s2k, sb))
                            qs = pB.tile([128, 512], f32, tag="qs")
                            nc.scalar.copy(qs[:], ps[:])
                            qsw = pB.tile([128, 512], f32, tag="qsw")
                            nc.vector.stream_shuffle(qsw[:], qs[:], SHUF)
                            m2 = pB.tile([128, 512], f32, tag="m2")
                            nc.gpsimd.tensor_mul(m2[:], qsw[:], st_t[:])
                            qc = pB.tile([128, 512], f32, tag="qc")
                            nc.vector.tensor_mul(qc[:], ps[:], ct_t[:])
                            nc.vector.tensor_add(_sb(kT, sb), qc[:], m2[:])
                        # V projection + transpose to natural layout
                        wvt = pBw.tile([128, NHT, 128], f32r, tag="wB")
                        if L >= 2:
                            nc.sync.dma_start(
                                wvt[:],
                                wv.rearrange("(o p) n -> p o n", p=128))
                        for sb in range(NSB if L >= 2 else 0):
                            ps = pBp.tile([128, 512], f32, tag="proj")
                            for ht in range(NHT):
                                nc.tensor.matmul(
                                    ps[:], wvt[:, ht, :],
                                    _sb(hT[:, ht, :], sb),
                                    start=(ht == 0), stop=(ht == NHT - 1))
                            vts = pB.tile([128, 512], f32, tag="vts")
                            nc.scalar.copy(vts[:], ps[:])
                            for k4 in range(4):
                                pt = pBp.tile([128, 128], f32, tag="vtr")
                                nc.tensor.transpose(
                                    pt[:], vts[:, k4 * 128:(k4 + 1) * 128],
                                    id_t[:])
                                nc.scalar.copy(v_nat[:, sb * 4 + k4, :],
                                               pt[:])

                    # ------- Phase C: per-head Q proj + rope + attention ----
                    if True:
                        with tc.tile_pool(name="pq", bufs=1) as pq, \
                             tc.tile_pool(name="pC", bufs=2) as pC, \
                             tc.tile_pool(name="pCw", bufs=1) as pCw, \
                             tc.tile_pool(name="pCp", bufs=2,
                                          space="PSUM") as pCp, \
                             tc.tile_pool(name="pCo", bufs=1,
                                          space="PSUM") as pCo:
                            for h in range(QH if L >= 3 else 0):
                                qTh = pq.tile([128, S], f32r, tag="qTh")
                                wqt = pCw.tile([128, NHT, 128], f32r,
                                               tag="wq")
                                nc.sync.dma_start(
                                    wqt[:],
                                    wq.rearrange("(o p) n -> p o n", p=128)
                                      [:, :, h * 128:(h + 1) * 128])
                                for sb in range(NSB):
                                    ps = pCp.tile([128, 512], f32,
                                                  tag="proj2")
                                    for ht in range(NHT):
                                        nc.tensor.matmul(
                                            ps[:], wqt[:, ht, :],
                                            _sb(hT[:, ht, :], sb),
                                            start=(ht == 0),
                                            stop=(ht == NHT - 1))
                                    ct_t = pC.tile([128, 512], f32,
                                                   tag="ropeC", bufs=1)
                                    st_t = pC.tile([128, 512], f32,
                                                   tag="ropeS", bufs=1)
                                    nc.sync.dma_start(ct_t[:], _sb(cq, sb))
                                    nc.sync.dma_start(st_t[:], _sb(s2q, sb))
                                    qs = pC.tile([128, 512], f32, tag="qs2", bufs=1)
                                    nc.scalar.copy(qs[:], ps[:])
                                    qsw = pC.tile([128, 512], f32,
                                                  tag="qsw2", bufs=1)
                                    nc.vector.stream_shuffle(qsw[:], qs[:],
                                                             SHUF)
                                    m2 = pC.tile([128, 512], f32, tag="m22", bufs=1)
                                    nc.gpsimd.tensor_mul(m2[:], qsw[:],
                                                         st_t[:])
                                    qc = pC.tile([128, 512], f32, tag="qc2", bufs=1)
                                    nc.vector.tensor_mul(qc[:], ps[:],
                                                         ct_t[:])
                                    nc.vector.tensor_add(_sb(qTh, sb),
                                                         qc[:], m2[:])
                                # attention for this head
                                for qb in range(NSB):
                                    acc = pCo.tile([128, 512], f32,
                                                   tag="acc")
                                    den = pCo.tile([1, 512], f32, tag="den")
                                    nkt = 4 * (qb + 1)
                                    for kt in range(nkt):
                                        j = kt - qb * 4
                                        coloff = max(0, j) * 128
                                        ncols = 512 - coloff
                                        qs0 = qb * 512 + coloff
                                        sc = pCp.tile([128, 512], f32,
                                                      tag="sc")
                                        nc.tensor.matmul(
                                            sc[:, 0:ncols],
                                            kT[:, kt * 128:(kt + 1) * 128],
                                            qTh[:, qs0:qs0 + ncols],
                                            start=True, stop=True)
                                        P = pC.tile([128, 512], f32r,
                                                    tag="P", bufs=3)
                                        nc.scalar.activation(
                                            P[:, 0:ncols], sc[:, 0:ncols],
                                            AF.Exp)
                                        if j >= 0:
                                            nc.vector.tensor_mul(
                                                P[:, 0:128], P[:, 0:128],
                                                tri_t[:])
                                        nc.tensor.matmul(
                                            acc[:, coloff:512],
                                            v_nat[:, kt, :], P[:, 0:ncols],
                                            start=(kt == 0),
                                            stop=(kt == nkt - 1))
                                        nc.tensor.matmul(
                                            den[0:1, coloff:512], ones_t[:],
                                            P[:, 0:ncols],
                                            start=(kt == 0),
                                            stop=(kt == nkt - 1))
                                    rd = pC.tile([1, 512], f32, tag="rd")
                                    nc.vector.reciprocal(rd[:], den[:])
                                    rb = pC.tile([128, 512], f32, tag="rb2")
                                    nc.gpsimd.partition_broadcast(rb[:],
                                                                  rd[:])
                                    ot = pC.tile([128, 512], f32r,
                                                 tag="ot")
                                    nc.vector.tensor_mul(ot[:], acc[:],
                                                         rb[:])
                                    nc.sync.dma_start(
                                        _sb(outd[h, :, :], qb), ot[:])

                        # ---- Phase D: Wo partial + chunked AllReduce ----
                        with tc.tile_pool(name="pD", bufs=2) as pD, \
                             tc.tile_pool(name="pDw", bufs=1) as pDw, \
                             tc.tile_pool(name="pDp", bufs=2,
                                          space="PSUM") as pDp:
                            wo_t = pDw.tile([128, QH, NHT, 128], f32r)
                            if L >= 4:
                                for k2 in range(QH):
                                    nc.sync.dma_start(
                                        wo_t[:, k2, :, :].rearrange(
                                            "p a b -> p (a b)"),
                                        wo[k2 * 128:(k2 + 1) * 128, :])
                            for sb in range(NSB if L >= 4 else 0):
                                osb = pD.tile([128, QH, 512], f32r,
                                              tag="osb", bufs=1)
                                nc.sync.dma_start(
                                    osb[:],
                                    outd[:, :, sb * 512:(sb + 1) * 512]
                                    .rearrange("o p n -> p o n"))
                                for ocg in range(2):
                                    xqg = pD.tile([128, 8, 512], f32,
                                                  tag="xqg", bufs=1)
                                    nc.sync.dma_start(
                                        xqg[:],
                                        xT.rearrange("(a p) n -> p a n",
                                                     p=128)
                                        [:, ocg * 8:(ocg + 1) * 8,
                                         sb * 512:(sb + 1) * 512])
                                    for oc8 in range(8):
                                        oc = ocg * 8 + oc8
                                        ps = pDp.tile([128, 512], f32,
                                                      tag="y")
                                        for k2 in range(QH):
                                            nc.tensor.matmul(
                                                ps[:],
                                                wo_t[:, k2, oc, :],
                                                osb[:, k2, :],
                                                start=(k2 == 0),
                                                stop=(k2 == QH - 1))
                                        yt = pD.tile([128, 512], f32,
                                                     tag="yt")
                                        nc.vector.scalar_tensor_tensor(
                                            out=yt[:], in0=xqg[:, oc8, :],
                                            scalar=0.25, in1=ps[:],
                                            op0=AL.mult, op1=AL.add)
                                        nc.sync.dma_start(
                                            ar_in[sb][oc * 128:
                                                      (oc + 1) * 128, :],
                                            yt[:])
                                if L >= 5:
                                    nc.gpsimd.collective_compute(
                                        "AllReduce", AL.add,
                                        replica_groups=GROUPS,
                                        ins=[ar_in[sb].opt()],
                                        outs=[ar_out[sb].opt()])

            # ---------- Phase E: x1 = xT + ar; rmsnorm2 -> h2T ----------
            with tc.tile_pool(name="ph2", bufs=1) as ph2:
                h2T = ph2.tile([128, NHT, S], f32r)
                with tc.tile_pool(name="pE", bufs=1) as pE, \
                     tc.tile_pool(name="pEs", bufs=2) as pEs, \
                     tc.tile_pool(name="pEp", bufs=2, space="PSUM") as pEp:
                    for sb in range(NSB if L >= 6 else 0):
                        x1sb = pE.tile([128, NHT, 512], f32, tag="x1sb")
                        ss_ps = pEp.tile([1, 512], f32, tag="ss2")
                        for ht in range(NHT):
                            nc.sync.dma_start(
                                x1sb[:, ht, :],
                                ar_out[sb][ht * 128:(ht + 1) * 128, :])
                            sq = pEs.tile([128, 512], f32r, tag="sq2")
                            nc.scalar.activation(sq[:], x1sb[:, ht, :],
                                                 AF.Square)
                            nc.tensor.matmul(ss_ps[:], ones_t[:], sq[:],
                                             start=(ht == 0),
                                             stop=(ht == NHT - 1))
                        sd = pEs.tile([1, 512], f32, tag="sd2")
                        nc.scalar.activation(sd[:], ss_ps[:], AF.Sqrt,
                                             bias=eps_t[0:1, :],
                                             scale=1.0 / H)
                        rr = pEs.tile([1, 512], f32, tag="rr2")
                        nc.vector.reciprocal(rr[:], sd[:])
                        rb = pEs.tile([128, 512], f32, tag="rb3")
                        nc.gpsimd.partition_broadcast(rb[:], rr[:])
                        for ht in range(NHT):
                            nc.vector.scalar_tensor_tensor(
                                out=_sb(h2T[:, ht, :], sb),
                                in0=x1sb[:, ht, :],
                                scalar=wn2_t[:, ht:ht + 1],
                                in1=rb[:], op0=AL.mult, op1=AL.mult)

                # ---------- Phase F1: gate/up/silu-mul -> mT (DRAM) -------
                with tc.tile_pool(name="pF", bufs=2) as pF, \
                     tc.tile_pool(name="pFw", bufs=2) as pFw, \
                     tc.tile_pool(name="pFp", bufs=2, space="PSUM") as pFp:
                    for ct in range(FCT if L >= 7 else 0):
                        wgt = pFw.tile([128, NHT, 128], f32r, tag="wg")
                        wut = pFw.tile([128, NHT, 128], f32r, tag="wu")
                        nc.sync.dma_start(
                            wgt[:], wg.rearrange("(o p) n -> p o n", p=128)
                                      [:, :, ct * 128:(ct + 1) * 128])
                        nc.sync.dma_start(
                            wut[:], wu.rearrange("(o p) n -> p o n", p=128)
                                      [:, :, ct * 128:(ct + 1) * 128])
                        for sb in range(NSB):
                            pg = pFp.tile([128, 512], f32, tag="pg")
                            pu = pFp.tile([128, 512], f32, tag="pu")
                            for ht in range(NHT):
                                nc.tensor.matmul(
                                    pg[:], wgt[:, ht, :],
                                    _sb(h2T[:, ht, :], sb),
                                    start=(ht == 0), stop=(ht == NHT - 1))
                            for ht in range(NHT):
                                nc.tensor.matmul(
                                    pu[:], wut[:, ht, :],
                                    _sb(h2T[:, ht, :], sb),
                                    start=(ht == 0), stop=(ht == NHT - 1))
                            sg = pF.tile([128, 512], f32, tag="sg")
                            nc.scalar.activation(sg[:], pg[:], AF.Silu)
                            mt = pF.tile([128, 512], f32r, tag="mt")
                            nc.vector.tensor_mul(mt[:], pu[:], sg[:])
                            nc.sync.dma_start(
                                _sb(mTd[ct, :, :], sb), mt[:])

            # ---------- Phase F2: down + 0.25*x1 -> chunked RS --------
            with tc.tile_pool(name="pwd", bufs=1) as pwd, \
                 tc.tile_pool(name="pGm", bufs=1) as pGm, \
                 tc.tile_pool(name="pG", bufs=2) as pG, \
                 tc.tile_pool(name="pGp", bufs=2, space="PSUM") as pGp:
                mm = pGm.tile([128, FCT, S], f32r)
                for ct in range(FCT if L >= 8 else 0):
                    nc.sync.dma_start(
                        mm[:, ct, :], mTd[ct, :, :])
                for oc in range(NHT if L >= 8 else 0):
                    wdo = pwd.tile([128, FCT, 128], f32r, tag="wdo",
                                   bufs=2)
                    nc.sync.dma_start(
                        wdo[:],
                        wd.rearrange("(a p) n -> p a n", p=128)
                        [:, :, oc * 128:(oc + 1) * 128])
                    for sb in range(NSB):
                        ps = pGp.tile([128, 512], f32, tag="pd")
                        for ct in range(FCT):
                            nc.tensor.matmul(
                                ps[:], wdo[:, ct, :],
                                mm[:, ct, sb * 512:(sb + 1) * 512],
                                start=(ct == 0), stop=(ct == FCT - 1))
                        x1t = pG.tile([128, 512], f32, tag="x1t")
                        nc.sync.dma_start(
                            x1t[:],
                            ar_out[sb][oc * 128:(oc + 1) * 128, :])
                        yd = pG.tile([128, 512], f32, tag="yd")
                        nc.vector.scalar_tensor_tensor(
                            out=yd[:], in0=x1t[:], scalar=0.25,
                            in1=ps[:], op0=AL.mult, op1=AL.add)
                        nc.sync.dma_start(
                            rs_in[oc // 8, sb,
                                  (oc % 8) * 128:(oc % 8 + 1) * 128, :],
                            yd[:])
                    if L >= 9 and oc % 8 == 7:
                        hh = oc // 8
                        nc.gpsimd.collective_compute(
                            "ReduceScatter", AL.add, replica_groups=GROUPS,
                            ins=[rs_in[hh].opt()],
                            outs=[rs_out[hh * 1024:(hh + 1) * 1024, :]
                                  .opt()])

            # ---------- Phase G: write output ----------
            if L >= 10:
                nc.sync.dma_start(outsl[:], rs_out[:])

    nc.finalize()
    return nc


_CACHE = {}


def _get_nc():
    if "nc" not in _CACHE:
        _CACHE["nc"] = build()
    return _CACHE["nc"]


def _host_prep(inputs):
    """Build the 8 per-core input maps from the full problem inputs."""
    x = np.asarray(inputs["x"], np.float32)
    Wq = np.asarray(inputs["Wq"], np.float32)
    Wk = np.asarray(inputs["Wk"], np.float32)
    Wv = np.asarray(inputs["Wv"], np.float32)
    Wo = np.asarray(inputs["Wo"], np.float32)
    Wg = np.asarray(inputs["Wgate"], np.float32)
    Wu = np.asarray(inputs["Wup"], np.float32)
    Wd = np.asarray(inputs["Wdown"], np.float32)
    wn1v = np.asarray(inputs["w_norm1"], np.float32)
    wn2v = np.asarray(inputs["w_norm2"], np.float32)
    cos = np.asarray(inputs["freqs_cos"], np.float32)
    sin = np.asarray(inputs["freqs_sin"], np.float32)

    scale = 1.0 / float(np.sqrt(np.float32(HD)))
    Cq, S2q = make_rope_tables(cos, sin, scale)
    Ck, S2k = make_rope_tables(cos, sin, 1.0)
    tri_np = (np.arange(128)[None, :] >= np.arange(128)[:, None])
    tri_np = tri_np.astype(np.float32)
    wn1_np = np.ascontiguousarray(wn1v.reshape(NHT, 128).T)
    wn2_np = np.ascontiguousarray(wn2v.reshape(NHT, 128).T)
    ones_np = np.ones((128, 1), np.float32)
    id_np = np.eye(128, dtype=np.float32)

    shared = dict(cq=Cq, s2q=S2q, ck=Ck, s2k=S2k, wn1=wn1_np, wn2=wn2_np,
                  tri=tri_np, ones=ones_np, ident=id_np,
                  epsb=np.full((128, 1), EPS, np.float32))

    per_tp = []
    for tp in range(TPN):
        qcols = []
        for h in range(tp * QH, (tp + 1) * QH):
            qcols.extend(h * HD + PERM)
        per_tp.append(dict(
            wq=round_fp32r(Wq[:, qcols]),
            wk=round_fp32r(Wk[:, tp * HD + PERM]),
            wv=round_fp32r(np.ascontiguousarray(
                Wv[:, tp * HD:(tp + 1) * HD])),
            wo=round_fp32r(np.ascontiguousarray(
                Wo[tp * QH * HD:(tp + 1) * QH * HD, :])),
            wg=round_fp32r(np.ascontiguousarray(
                Wg[:, tp * FFS:(tp + 1) * FFS])),
            wu=round_fp32r(np.ascontiguousarray(
                Wu[:, tp * FFS:(tp + 1) * FFS])),
            wd=round_fp32r(np.ascontiguousarray(
                Wd[tp * FFS:(tp + 1) * FFS, :])),
        ))

    xTb = [np.ascontiguousarray(x[dp].T) for dp in range(2)]
    in_maps = []
    for c in range(8):
        dp, tp = c // 4, c % 4
        m = dict(shared)
        m.update(per_tp[tp])
        m["xT"] = xTb[dp]
        in_maps.append(m)
    return in_maps


def kernel(**inputs) -> np.ndarray:
    nc = _get_nc()
    in_maps = _host_prep(inputs)
    res = run_bass_kernel_spmd(nc, in_maps, core_ids=list(range(8)),
                               trace=False)
    out = np.zeros((B, S, H), np.float32)
    for c in range(8):
        dp, tp = c // 4, c % 4
        sl = res.results[c]["outsl"]          # [H, 512]
        out[dp, tp * SSL:(tp + 1) * SSL, :] = sl.T
    return out



# revision 6
# speedup vs baseline: 8.0285x; 8.0285x over previous
"""Trainium2 Bass kernel for nn_MiniAgentBlock (dense transformer block).

Sharding: DP=2 over batch x TP=4 within each batch (8 NeuronCores).
Core c: dp = c//4 (batch), tp = c%4 (4 q-heads / 1 kv-head, FF/4 slice).

Wall-clock optimizations over the first working version:
- The jitted shard_map executable is built ONCE and cached; repeat calls
  skip jax re-trace / XLA+neuronxcc re-compile / NEFF reload.
- All large inputs ship as fp16 (error budget: rel tol 2e-2, fp16
  quantization contributes ~1e-3).
- No duplicated bytes over the (slow, ~40MB/s) axon tunnel:
  x ships as per-core [512, S] H-shards, AllGathered on device across
  the TP group; every weight ships as a half split along its input dim
  across the DP pair (cores c, c+4 hold the same TP slice), AllGathered
  on device across pair groups [[0,4],[1,5],[2,6],[3,7]].
- Rope tables ship as compact [64, S] cos/sin, expanded on device into
  SBUF; the 1/sqrt(HD) score scale is folded into the Exp activation.
- Output returns as fp16 [H, S/4] per core.

Device kernel: all matmul phases run in transposed [feature, seq]
layout; projection/FFN matmuls in fp16 (full PE rate), attention in
fp32r. On-device AllReduce after the attention output projection and
ReduceScatter after the FFN down projection, within each 4-core group.
The residual x1 = x + attn is folded into the ReduceScatter as 0.25*x1
per core, so the program is identical on every core (pure SPMD).
"""
import sys
if "/opt/trn_rl_repo" not in sys.path:
    sys.path.insert(0, "/opt/trn_rl_repo")

import numpy as np
import concourse.bass as bass
import concourse.mybir as mybir
import concourse.tile as tile
from concourse import bacc

f32 = mybir.dt.float32
f32r = mybir.dt.float32r
f16 = mybir.dt.float16
AL = mybir.AluOpType
AF = mybir.ActivationFunctionType

B, S, H = 2, 2048, 2048
NH, NKV, HD = 16, 4, 128
FF = 5632
EPS = 1e-5
TPN = 4
QH = NH // TPN           # 4 q heads per core
FFS = FF // TPN          # 1408
FCT = FFS // 128         # 11 FF col tiles
SSL = S // TPN           # 512 output seq cols per core
NHT = H // 128           # 16 H tiles
NST = S // 128           # 16 seq tiles
NSB = S // 512           # 4 seq blocks
GROUPS = [[0, 1, 2, 3], [4, 5, 6, 7]]
PAIRS = [[0, 4], [1, 5], [2, 6], [3, 7]]
SCALE = 1.0 / float(np.sqrt(np.float32(HD)))

# HD permutation: quadrant q: [evens 16q..16q+15 | odds 16q..16q+15]
PERM = np.zeros(HD, dtype=np.int64)
for _q in range(4):
    for _i in range(16):
        PERM[32 * _q + _i] = 2 * (16 * _q + _i)
        PERM[32 * _q + 16 + _i] = 2 * (16 * _q + _i) + 1
SHUF = [(i + 16) % 32 for i in range(32)]


def _sb(x, sb):
    return x[:, sb * 512:(sb + 1) * 512]


def build():
    nc = bacc.Bacc("TRN2", target_bir_lowering=False, debug=False,
                   num_devices=8)

    def din(name, shape, dt=f16):
        return nc.dram_tensor(name, list(shape), dt, kind="ExternalInput")

    xs = din("xs", [512, S])               # H-shard of x[dp].T
    wqh = din("wqh", [1024, TPN * HD])     # permuted cols, row half
    wkh = din("wkh", [1024, HD])           # permuted cols, row half
    wvh = din("wvh", [1024, HD])
    woh = din("woh", [256, H])
    wgh = din("wgh", [1024, FFS])
    wuh = din("wuh", [1024, FFS])
    wdh = din("wdh", [704, H])
    cosT = din("cosT", [64, S], f32)       # cos(ang).T
    sinT = din("sinT", [64, S], f32)
    wn1 = din("wn1", [128, NHT], f32)      # w_norm1[ht*128+p] at [p, ht]
    wn2 = din("wn2", [128, NHT], f32)
    tri = din("tri", [128, 128], f32r)     # tri[k,i] = (i >= k)
    ones = din("ones", [128, 1], f32r)
    epsb = din("epsb", [128, 1], f32)      # EPS bias tile
    ident = din("ident", [128, 128], f32)  # f32 identity
    outsl = nc.dram_tensor("outsl", [H, SSL], f16, kind="ExternalOutput")

    with tile.TileContext(nc) as tc:
        with tc.tile_pool(name="pconst", bufs=1) as pconst, \
             tc.tile_pool(name="pdram", bufs=1, space="DRAM") as pdram:
            ones_t = pconst.tile([128, 1], f32r)
            tri_t = pconst.tile([128, 128], f32r)
            id_t = pconst.tile([128, 128], f32)
            wn1_t = pconst.tile([128, NHT], f32)
            wn2_t = pconst.tile([128, NHT], f32)
            eps_t = pconst.tile([128, 1], f32)
            ctab = pconst.tile([128, S], f32)
            stab = pconst.tile([128, S], f32)
            sT = pconst.tile([64, S], f32)
            nc.sync.dma_start(ones_t[:], ones[:])
            nc.sync.dma_start(tri_t[:], tri[:])
            nc.sync.dma_start(id_t[:], ident[:])
            nc.sync.dma_start(wn1_t[:], wn1[:])
            nc.sync.dma_start(wn2_t[:], wn2[:])
            nc.sync.dma_start(eps_t[:], epsb[:])
            nc.sync.dma_start(sT[:], sinT[:])
            # rope tables: ctab[32q+i] = ctab[32q+16+i] = cos[:, 16q+i]
            #              stab[32q+i] = -sin, stab[32q+16+i] = +sin
            # (engine ops need 32-aligned partition bases; negate once at
            #  partition 0 and bounce via DRAM, then DMA rows into place)
            snegs = pconst.tile([64, S], f32)
            nc.scalar.activation(snegs[:], sT[:], AF.Copy, scale=-1.0)
            for q in range(4):
                nc.sync.dma_start(ctab[32 * q:32 * q + 16, :],
                                  cosT[16 * q:16 * q + 16, :])
                nc.sync.dma_start(ctab[32 * q + 16:32 * q + 32, :],
                                  cosT[16 * q:16 * q + 16, :])
                nc.sync.dma_start(stab[32 * q + 16:32 * q + 32, :],
                                  sinT[16 * q:16 * q + 16, :])

            # DRAM scratch
            snegd = pdram.tile([64, S], f32)
            nc.sync.dma_start(snegd[:], snegs[:])
            for q in range(4):
                nc.sync.dma_start(stab[32 * q:32 * q + 16, :],
                                  snegd[16 * q:16 * q + 16, :])
            xg = pdram.tile([H, S], f16)
            wqf = pdram.tile([H, TPN * HD], f16)
            wkf = pdram.tile([H, HD], f16)
            wvf = pdram.tile([H, HD], f16)
            wof = pdram.tile([QH * HD, H], f16)
            wgf = pdram.tile([H, FFS], f16)
            wuf = pdram.tile([H, FFS], f16)
            wdf = pdram.tile([FFS, H], f16)
            outd = pdram.tile([QH, 128, S], f16)
            ar_in = [pdram.tile([H, 512], f32, name=f"ar_in{i}")
                     for i in range(NSB)]
            ar_out = [pdram.tile([H, 512], f32, name=f"ar_out{i}")
                      for i in range(NSB)]
            mTd = pdram.tile([FCT, 128, S], f16)
            rs_in = pdram.tile([2, NSB, 1024, 512], f16)  # [hh, sb, r, c]
            rs_out = pdram.tile([H, 512], f16)

            # ---------- Phase 0: materialize full x / weights on device ----
            # (collectives cannot read IO tensors; stage via internal DRAM)
            xs_st = pdram.tile([512, S], f16)
            nc.sync.dma_start(xs_st[:], xs[:])
            nc.gpsimd.collective_compute(
                "AllGather", AL.bypass, replica_groups=GROUPS,
                ins=[xs_st[:].opt()], outs=[xg[:].opt()])
            for (src, dst) in ((wkh, wkf), (wvh, wvf), (wqh, wqf),
                               (woh, wof), (wgh, wgf), (wuh, wuf),
                               (wdh, wdf)):
                st = pdram.tile(list(src.shape), f16,
                                name=f"st_{src.name}")
                nc.sync.dma_start(st[:], src[:])
                nc.gpsimd.collective_compute(
                    "AllGather", AL.bypass, replica_groups=PAIRS,
                    ins=[st[:].opt()], outs=[dst[:].opt()])

            with tc.tile_pool(name="phT", bufs=1) as phT:
                hT = phT.tile([128, NHT, S], f16)

                # ---------- Phase A: rmsnorm1 -> hT ----------
                with tc.tile_pool(name="pA", bufs=1) as pA, \
                     tc.tile_pool(name="pAs", bufs=2) as pAs, \
                     tc.tile_pool(name="pAp", bufs=2, space="PSUM") as pAp:
                    for sb in range(NSB):
                        xsb = pA.tile([128, NHT, 512], f16, tag="xsb")
                        ss_ps = pAp.tile([1, 512], f32, tag="ss")
                        for ht in range(NHT):
                            nc.sync.dma_start(
                                xsb[:, ht, :],
                                _sb(xg[ht * 128:(ht + 1) * 128, :], sb))
                            sq = pAs.tile([128, 512], f32r, tag="sq")
                            nc.scalar.activation(sq[:], xsb[:, ht, :],
                                                 AF.Square)
                            nc.tensor.matmul(ss_ps[:], ones_t[:], sq[:],
                                             start=(ht == 0),
                                             stop=(ht == NHT - 1))
                        sd = pAs.tile([1, 512], f32, tag="sd")
                        nc.scalar.activation(sd[:], ss_ps[:], AF.Sqrt,
                                             bias=eps_t[0:1, :],
                                             scale=1.0 / H)
                        rr = pAs.tile([1, 512], f32, tag="rr")
                        nc.vector.reciprocal(rr[:], sd[:])
                        rb = pAs.tile([128, 512], f32, tag="rb")
                        nc.gpsimd.partition_broadcast(rb[:], rr[:])
                        for ht in range(NHT):
                            nc.vector.scalar_tensor_tensor(
                                out=_sb(hT[:, ht, :], sb),
                                in0=xsb[:, ht, :],
                                scalar=wn1_t[:, ht:ht + 1],
                                in1=rb[:], op0=AL.mult, op1=AL.mult)

                # ---------- Phase B: K/V projections + K rope ----------
                with tc.tile_pool(name="pkv", bufs=1) as pkv:
                    kT = pkv.tile([128, S], f32r)
                    v_nat = pkv.tile([128, NST, HD], f32r)

                    with tc.tile_pool(name="pB", bufs=1) as pB, \
                         tc.tile_pool(name="pBw", bufs=1) as pBw, \
                         tc.tile_pool(name="pBp", bufs=2,
                                      space="PSUM") as pBp:
                        wkt = pBw.tile([128, NHT, 128], f16, tag="wB")
                        nc.sync.dma_start(
                            wkt[:],
                            wkf[:].rearrange("(o p) n -> p o n", p=128))
                        for sb in range(NSB):
                            ps = pBp.tile([128, 512], f32, tag="proj")
                            for ht in range(NHT):
                                nc.tensor.matmul(
                                    ps[:], wkt[:, ht, :],
                                    _sb(hT[:, ht, :], sb),
                                    start=(ht == 0), stop=(ht == NHT - 1))
                            qs = pB.tile([128, 512], f32, tag="qs")
                            nc.scalar.copy(qs[:], ps[:])
                            qsw = pB.tile([128, 512], f32, tag="qsw")
                            nc.vector.stream_shuffle(qsw[:], qs[:], SHUF)
                            m2 = pB.tile([128, 512], f32, tag="m2")
                            nc.gpsimd.tensor_mul(m2[:], qsw[:],
                                                 _sb(stab, sb))
                            qc = pB.tile([128, 512], f32, tag="qc")
                            nc.vector.tensor_mul(qc[:], ps[:],
                                                 _sb(ctab, sb))
                            nc.vector.tensor_add(_sb(kT, sb), qc[:], m2[:])
                        # V projection + transpose to natural layout
                        wvt = pBw.tile([128, NHT, 128], f16, tag="wB")
                        nc.sync.dma_start(
                            wvt[:],
                            wvf[:].rearrange("(o p) n -> p o n", p=128))
                        for sb in range(NSB):
                            ps = pBp.tile([128, 512], f32, tag="proj")
                            for ht in range(NHT):
                                nc.tensor.matmul(
                                    ps[:], wvt[:, ht, :],
                                    _sb(hT[:, ht, :], sb),
                                    start=(ht == 0), stop=(ht == NHT - 1))
                            vts = pB.tile([128, 512], f32, tag="vts")
                            nc.scalar.copy(vts[:], ps[:])
                            for k4 in range(4):
                                pt = pBp.tile([128, 128], f32, tag="vtr")
                                nc.tensor.transpose(
                                    pt[:], vts[:, k4 * 128:(k4 + 1) * 128],
                                    id_t[:])
                                nc.scalar.copy(v_nat[:, sb * 4 + k4, :],
                                               pt[:])

                    # ------- Phase C: per-head Q proj + rope + attention ----
                    with tc.tile_pool(name="pq", bufs=1) as pq, \
                         tc.tile_pool(name="pC", bufs=2) as pC, \
                         tc.tile_pool(name="pCw", bufs=1) as pCw, \
                         tc.tile_pool(name="pCp", bufs=2,
                                      space="PSUM") as pCp, \
                         tc.tile_pool(name="pCo", bufs=1,
                                      space="PSUM") as pCo:
                        for h in range(QH):
                            qTh = pq.tile([128, S], f32r, tag="qTh")
                            wqt = pCw.tile([128, NHT, 128], f16, tag="wq")
                            nc.sync.dma_start(
                                wqt[:],
                                wqf[:].rearrange("(o p) n -> p o n", p=128)
                                   [:, :, h * 128:(h + 1) * 128])
                            for sb in range(NSB):
                                ps = pCp.tile([128, 512], f32, tag="proj2")
                                for ht in range(NHT):
                                    nc.tensor.matmul(
                                        ps[:], wqt[:, ht, :],
                                        _sb(hT[:, ht, :], sb),
                                        start=(ht == 0),
                                        stop=(ht == NHT - 1))
                                qs = pC.tile([128, 512], f32, tag="qs2",
                                             bufs=1)
                                nc.scalar.copy(qs[:], ps[:])
                                qsw = pC.tile([128, 512], f32, tag="qsw2",
                                              bufs=1)
                                nc.vector.stream_shuffle(qsw[:], qs[:],
                                                         SHUF)
                                m2 = pC.tile([128, 512], f32, tag="m22",
                                             bufs=1)
                                nc.gpsimd.tensor_mul(m2[:], qsw[:],
                                                     _sb(stab, sb))
                                qc = pC.tile([128, 512], f32, tag="qc2",
                                             bufs=1)
                                nc.vector.tensor_mul(qc[:], ps[:],
                                                     _sb(ctab, sb))
                                nc.vector.tensor_add(_sb(qTh, sb),
                                                     qc[:], m2[:])
                            # attention for this head
                            for qb in range(NSB):
                                acc = pCo.tile([128, 512], f32, tag="acc")
                                den = pCo.tile([1, 512], f32, tag="den")
                                nkt = 4 * (qb + 1)
                                for kt in range(nkt):
                                    j = kt - qb * 4
                                    coloff = max(0, j) * 128
                                    ncols = 512 - coloff
                                    qs0 = qb * 512 + coloff
                                    sc = pCp.tile([128, 512], f32,
                                                  tag="sc")
                                    nc.tensor.matmul(
                                        sc[:, 0:ncols],
                                        kT[:, kt * 128:(kt + 1) * 128],
                                        qTh[:, qs0:qs0 + ncols],
                                        start=True, stop=True)
                                    P = pC.tile([128, 512], f32r,
                                                tag="P", bufs=3)
                                    nc.scalar.activation(
                                        P[:, 0:ncols], sc[:, 0:ncols],
                                        AF.Exp, scale=SCALE)
                                    if j >= 0:
                                        nc.vector.tensor_mul(
                                            P[:, 0:128], P[:, 0:128],
                                            tri_t[:])
                                    nc.tensor.matmul(
                                        acc[:, coloff:512],
                                        v_nat[:, kt, :], P[:, 0:ncols],
                                        start=(kt == 0),
                                        stop=(kt == nkt - 1))
                                    nc.tensor.matmul(
                                        den[0:1, coloff:512], ones_t[:],
                                        P[:, 0:ncols],
                                        start=(kt == 0),
                                        stop=(kt == nkt - 1))
                                rd = pC.tile([1, 512], f32, tag="rd")
                                nc.vector.reciprocal(rd[:], den[:])
                                rb = pC.tile([128, 512], f32, tag="rb2")
                                nc.gpsimd.partition_broadcast(rb[:], rd[:])
                                ot = pC.tile([128, 512], f16, tag="ot")
                                nc.vector.tensor_mul(ot[:], acc[:], rb[:])
                                nc.sync.dma_start(
                                    _sb(outd[h, :, :], qb), ot[:])

                    # ---- Phase D: Wo partial + chunked AllReduce ----
                    with tc.tile_pool(name="pD", bufs=2) as pD, \
                         tc.tile_pool(name="pDw", bufs=1) as pDw, \
                         tc.tile_pool(name="pDp", bufs=2,
                                      space="PSUM") as pDp:
                        wo_t = pDw.tile([128, QH, NHT, 128], f16)
                        for k2 in range(QH):
                            nc.sync.dma_start(
                                wo_t[:, k2, :, :].rearrange(
                                    "p a b -> p (a b)"),
                                wof[k2 * 128:(k2 + 1) * 128, :])
                        for sb in range(NSB):
                            osb = pD.tile([128, QH, 512], f16,
                                          tag="osb", bufs=1)
                            nc.sync.dma_start(
                                osb[:],
                                outd[:, :, sb * 512:(sb + 1) * 512]
                                .rearrange("o p n -> p o n"))
                            for ocg in range(2):
                                xqg = pD.tile([128, 8, 512], f16,
                                              tag="xqg", bufs=1)
                                nc.sync.dma_start(
                                    xqg[:],
                                    xg[:].rearrange("(a p) n -> p a n",
                                                    p=128)
                                    [:, ocg * 8:(ocg + 1) * 8,
                                     sb * 512:(sb + 1) * 512])
                                for oc8 in range(8):
                                    oc = ocg * 8 + oc8
                                    ps = pDp.tile([128, 512], f32,
                                                  tag="y")
                                    for k2 in range(QH):
                                        nc.tensor.matmul(
                                            ps[:],
                                            wo_t[:, k2, oc, :],
                                            osb[:, k2, :],
                                            start=(k2 == 0),
                                            stop=(k2 == QH - 1))
                                    yt = pD.tile([128, 512], f32,
                                                 tag="yt")
                                    nc.vector.scalar_tensor_tensor(
                                        out=yt[:], in0=xqg[:, oc8, :],
                                        scalar=0.25, in1=ps[:],
                                        op0=AL.mult, op1=AL.add)
                                    nc.sync.dma_start(
                                        ar_in[sb][oc * 128:
                                                  (oc + 1) * 128, :],
                                        yt[:])
                            nc.gpsimd.collective_compute(
                                "AllReduce", AL.add,
                                replica_groups=GROUPS,
                                ins=[ar_in[sb].opt()],
                                outs=[ar_out[sb].opt()])

            # ---------- Phase E: x1 = xT + ar; rmsnorm2 -> h2T ----------
            with tc.tile_pool(name="ph2", bufs=1) as ph2:
                h2T = ph2.tile([128, NHT, S], f16)
                with tc.tile_pool(name="pE", bufs=1) as pE, \
                     tc.tile_pool(name="pEs", bufs=2) as pEs, \
                     tc.tile_pool(name="pEp", bufs=2, space="PSUM") as pEp:
                    for sb in range(NSB):
                        x1sb = pE.tile([128, NHT, 512], f32, tag="x1sb")
                        ss_ps = pEp.tile([1, 512], f32, tag="ss2")
                        for ht in range(NHT):
                            nc.sync.dma_start(
                                x1sb[:, ht, :],
                                ar_out[sb][ht * 128:(ht + 1) * 128, :])
                            sq = pEs.tile([128, 512], f32r, tag="sq2")
                            nc.scalar.activation(sq[:], x1sb[:, ht, :],
                                                 AF.Square)
                            nc.tensor.matmul(ss_ps[:], ones_t[:], sq[:],
                                             start=(ht == 0),
                                             stop=(ht == NHT - 1))
                        sd = pEs.tile([1, 512], f32, tag="sd2")
                        nc.scalar.activation(sd[:], ss_ps[:], AF.Sqrt,
                                             bias=eps_t[0:1, :],
                                             scale=1.0 / H)
                        rr = pEs.tile([1, 512], f32, tag="rr2")
                        nc.vector.reciprocal(rr[:], sd[:])
                        rb = pEs.tile([128, 512], f32, tag="rb3")
                        nc.gpsimd.partition_broadcast(rb[:], rr[:])
                        for ht in range(NHT):
                            nc.vector.scalar_tensor_tensor(
                                out=_sb(h2T[:, ht, :], sb),
                                in0=x1sb[:, ht, :],
                                scalar=wn2_t[:, ht:ht + 1],
                                in1=rb[:], op0=AL.mult, op1=AL.mult)

                # ---------- Phase F1: gate/up/silu-mul -> mT (DRAM) -------
                with tc.tile_pool(name="pF", bufs=2) as pF, \
                     tc.tile_pool(name="pFw", bufs=2) as pFw, \
                     tc.tile_pool(name="pFp", bufs=2, space="PSUM") as pFp:
                    for ct in range(FCT):
                        wgt = pFw.tile([128, NHT, 128], f16, tag="wg")
                        wut = pFw.tile([128, NHT, 128], f16, tag="wu")
                        nc.sync.dma_start(
                            wgt[:],
                            wgf[:].rearrange("(o p) n -> p o n", p=128)
                               [:, :, ct * 128:(ct + 1) * 128])
                        nc.sync.dma_start(
                            wut[:],
                            wuf[:].rearrange("(o p) n -> p o n", p=128)
                               [:, :, ct * 128:(ct + 1) * 128])
                        for sb in range(NSB):
                            pg = pFp.tile([128, 512], f32, tag="pg")
                            pu = pFp.tile([128, 512], f32, tag="pu")
                            for ht in range(NHT):
                                nc.tensor.matmul(
                                    pg[:], wgt[:, ht, :],
                                    _sb(h2T[:, ht, :], sb),
                                    start=(ht == 0), stop=(ht == NHT - 1))
                            for ht in range(NHT):
                                nc.tensor.matmul(
                                    pu[:], wut[:, ht, :],
                                    _sb(h2T[:, ht, :], sb),
                                    start=(ht == 0), stop=(ht == NHT - 1))
                            sg = pF.tile([128, 512], f32, tag="sg")
                            nc.scalar.activation(sg[:], pg[:], AF.Silu)
                            mt = pF.tile([128, 512], f16, tag="mt")
                            nc.vector.tensor_mul(mt[:], pu[:], sg[:])
                            nc.sync.dma_start(
                                _sb(mTd[ct, :, :], sb), mt[:])

            # ---------- Phase F2: down + 0.25*x1 -> chunked RS --------
            with tc.tile_pool(name="pwd", bufs=1) as pwd, \
                 tc.tile_pool(name="pGm", bufs=1) as pGm, \
                 tc.tile_pool(name="pG", bufs=2) as pG, \
                 tc.tile_pool(name="pGp", bufs=2, space="PSUM") as pGp:
                mm = pGm.tile([128, FCT, S], f16)
                for ct in range(FCT):
                    nc.sync.dma_start(mm[:, ct, :], mTd[ct, :, :])
                for oc in range(NHT):
                    wdo = pwd.tile([128, FCT, 128], f16, tag="wdo",
                                   bufs=2)
                    nc.sync.dma_start(
                        wdo[:],
                        wdf[:].rearrange("(a p) n -> p a n", p=128)
                        [:, :, oc * 128:(oc + 1) * 128])
                    for sb in range(NSB):
                        ps = pGp.tile([128, 512], f32, tag="pd")
                        for ct in range(FCT):
                            nc.tensor.matmul(
                                ps[:], wdo[:, ct, :],
                                mm[:, ct, sb * 512:(sb + 1) * 512],
                                start=(ct == 0), stop=(ct == FCT - 1))
                        x1t = pG.tile([128, 512], f32, tag="x1t")
                        nc.sync.dma_start(
                            x1t[:],
                            ar_out[sb][oc * 128:(oc + 1) * 128, :])
                        yd = pG.tile([128, 512], f16, tag="yd")
                        nc.vector.scalar_tensor_tensor(
                            out=yd[:], in0=x1t[:], scalar=0.25,
                            in1=ps[:], op0=AL.mult, op1=AL.add)
                        nc.sync.dma_start(
                            rs_in[oc // 8, sb,
                                  (oc % 8) * 128:(oc % 8 + 1) * 128, :],
                            yd[:])
                    if oc % 8 == 7:
                        hh = oc // 8
                        nc.gpsimd.collective_compute(
                            "ReduceScatter", AL.add, replica_groups=GROUPS,
                            ins=[rs_in[hh].opt()],
                            outs=[rs_out[hh * 1024:(hh + 1) * 1024, :]
                                  .opt()])

            # ---------- Phase G: write output ----------
            nc.sync.dma_start(outsl[:], rs_out[:])

    nc.finalize()
    return nc


_CACHE = {}


def _get_nc():
    if "nc" not in _CACHE:
        _CACHE["nc"] = build()
    return _CACHE["nc"]


def _get_runner():
    """Build the jitted shard_map executable ONCE; reuse across calls."""
    if "runner" in _CACHE:
        return _CACHE["runner"]

    import jax
    from jax.sharding import Mesh, PartitionSpec
    from jax.experimental.shard_map import shard_map
    from concourse import bass2jax

    bass2jax.install_neuronx_cc_hook()
    nc = _get_nc()
    n_cores = 8

    partition_name = (nc.partition_id_tensor.name
                      if nc.partition_id_tensor else None)
    in_names, out_names, out_avals, zero_outs = [], [], [], []
    for alloc in nc.m.functions[0].allocations:
        if not isinstance(alloc, mybir.MemoryLocationSet):
            continue
        name = alloc.memorylocations[0].name
        if alloc.kind == "ExternalInput":
            if name != partition_name:
                in_names.append(name)
        elif alloc.kind == "ExternalOutput":
            shape = tuple(alloc.tensor_shape)
            dtype = mybir.dt.np(alloc.dtype)
            out_names.append(name)
            out_avals.append(jax.core.ShapedArray(shape, dtype))
            zero_outs.append(np.zeros(shape, dtype))
    n_params = len(in_names)
    n_outs = len(out_avals)
    all_names = list(in_names) + list(out_names)
    if partition_name is not None:
        all_names.append(partition_name)
    donate = tuple(range(n_params, n_params + n_outs))

    def _body(*args):
        operands = list(args)
        if partition_name is not None:
            operands.append(bass2jax.partition_id_tensor())
        outs = bass2jax._bass_exec_p.bind(
            *operands,
            out_avals=tuple(out_avals),
            in_names=tuple(all_names),
            out_names=tuple(out_names),
            lowering_input_output_aliases=(),
            sim_require_finite=True,
            sim_require_nnan=True,
            nc=nc,
        )
        return tuple(outs)

    devices = jax.devices()[:n_cores]
    mesh = Mesh(np.asarray(devices), ("core",))
    in_specs = (PartitionSpec("core"),) * (n_params + n_outs)
    out_specs = (PartitionSpec("core"),) * n_outs
    sharded = jax.jit(
        shard_map(_body, mesh=mesh, in_specs=in_specs,
                  out_specs=out_specs, check_rep=False),
        donate_argnums=donate,
        keep_unused=True,
    )
    concat_zeros = [
        np.zeros((n_cores * z.shape[0], *z.shape[1:]), z.dtype)
        for z in zero_outs
    ]

    def run(in_maps):
        concat_in = [
            np.concatenate([np.asarray(in_maps[c][name])
                            for c in range(n_cores)], axis=0)
            for name in in_names
        ]
        out_arrs = sharded(*concat_in, *concat_zeros)
        return [
            {name: np.asarray(out_arrs[i]).reshape(
                n_cores, *out_avals[i].shape)[c]
             for i, name in enumerate(out_names)}
            for c in range(n_cores)
        ]

    _CACHE["runner"] = run
    return run


def _host_prep(inputs):
    """Build the 8 per-core input maps from the full problem inputs."""
    x = np.asarray(inputs["x"], np.float32)
    Wq = np.asarray(inputs["Wq"], np.float32).astype(np.float16)
    Wk = np.asarray(inputs["Wk"], np.float32).astype(np.float16)
    Wv = np.asarray(inputs["Wv"], np.float32).astype(np.float16)
    Wo = np.asarray(inputs["Wo"], np.float32).astype(np.float16)
    Wg = np.asarray(inputs["Wgate"], np.float32).astype(np.float16)
    Wu = np.asarray(inputs["Wup"], np.float32).astype(np.float16)
    Wd = np.asarray(inputs["Wdown"], np.float32).astype(np.float16)
    wn1v = np.asarray(inputs["w_norm1"], np.float32)
    wn2v = np.asarray(inputs["w_norm2"], np.float32)
    cos = np.asarray(inputs["freqs_cos"], np.float32)
    sin = np.asarray(inputs["freqs_sin"], np.float32)

    tri_np = (np.arange(128)[None, :] >= np.arange(128)[:, None])
    tri_np = tri_np.astype(np.float32)
    wn1_np = np.ascontiguousarray(wn1v.reshape(NHT, 128).T)
    wn2_np = np.ascontiguousarray(wn2v.reshape(NHT, 128).T)

    shared = dict(cosT=np.ascontiguousarray(cos.T),
                  sinT=np.ascontiguousarray(sin.T),
                  wn1=wn1_np, wn2=wn2_np, tri=tri_np,
                  ones=np.ones((128, 1), np.float32),
                  ident=np.eye(128, dtype=np.float32),
                  epsb=np.full((128, 1), EPS, np.float32))

    halves = []      # halves[dp][tp] -> dict of weight-half arrays
    for dp in range(2):
        r0, r1 = dp * 1024, (dp + 1) * 1024
        per_tp = []
        for tp in range(TPN):
            qcols = []
            for h in range(tp * QH, (tp + 1) * QH):
                qcols.extend(h * HD + PERM)
            per_tp.append(dict(
                wqh=Wq[r0:r1][:, qcols],
                wkh=Wk[r0:r1][:, tp * HD + PERM],
                wvh=np.ascontiguousarray(
                    Wv[r0:r1, tp * HD:(tp + 1) * HD]),
                woh=np.ascontiguousarray(
                    Wo[tp * QH * HD + dp * 256:
                       tp * QH * HD + (dp + 1) * 256, :]),
                wgh=np.ascontiguousarray(
                    Wg[r0:r1, tp * FFS:(tp + 1) * FFS]),
                wuh=np.ascontiguousarray(
                    Wu[r0:r1, tp * FFS:(tp + 1) * FFS]),
                wdh=np.ascontiguousarray(
                    Wd[tp * FFS + dp * 704:tp * FFS + (dp + 1) * 704, :]),
            ))
        halves.append(per_tp)

    in_maps = []
    for c in range(8):
        dp, tp = c // 4, c % 4
        m = dict(shared)
        m.update(halves[dp][tp])
        m["xs"] = np.ascontiguousarray(
            x[dp][:, tp * 512:(tp + 1) * 512].T).astype(np.float16)
        in_maps.append(m)
    return in_maps


def kernel(**inputs) -> np.ndarray:
    run = _get_runner()
    in_maps = _host_prep(inputs)
    res = run(in_maps)
    out = np.zeros((B, S, H), np.float32)
    for c in range(8):
        dp, tp = c // 4, c % 4
        sl = res[c]["outsl"]                  # [H, 512] fp16
        out[dp, tp * SSL:(tp + 1) * SSL, :] = sl.T.astype(np.float32)
    return out


# revision 9
# speedup vs baseline: 26.7015x; 3.3258x over previous
"""Trainium2 Bass kernel for nn_MiniAgentBlock (dense transformer block).

Sharding: DP=2 over batch x TP=4 within each batch (8 NeuronCores).
Core c: dp = c//4 (batch), tp = c%4 (4 q-heads / 1 kv-head, FF/4 slice).

Wall-clock optimizations over the first working version:
- The jitted shard_map executable is built ONCE and cached; repeat calls
  skip jax re-trace / XLA+neuronxcc re-compile / NEFF reload.
- All large inputs ship as fp16 (error budget: rel tol 2e-2, fp16
  quantization contributes ~1e-3).
- No duplicated bytes over the (slow, ~40MB/s) axon tunnel:
  x ships as per-core [512, S] H-shards, AllGathered on device across
  the TP group; every weight ships as a half split along its input dim
  across the DP pair (cores c, c+4 hold the same TP slice), AllGathered
  on device across pair groups [[0,4],[1,5],[2,6],[3,7]].
- Rope tables ship as compact [64, S] cos/sin, expanded on device into
  SBUF; the 1/sqrt(HD) score scale is folded into the Exp activation.
- Output returns as fp16 [H, S/4] per core.

Device kernel: all matmul phases run in transposed [feature, seq]
layout; projection/FFN matmuls in fp16 (full PE rate), attention in
fp32r. On-device AllReduce after the attention output projection and
ReduceScatter after the FFN down projection, within each 4-core group.
The residual x1 = x + attn is folded into the ReduceScatter as 0.25*x1
per core, so the program is identical on every core (pure SPMD).
"""
import sys
if "/opt/trn_rl_repo" not in sys.path:
    sys.path.insert(0, "/opt/trn_rl_repo")

import numpy as np
import concourse.bass as bass
import concourse.mybir as mybir
import concourse.tile as tile
from concourse import bacc

f32 = mybir.dt.float32
f32r = mybir.dt.float32r
f16 = mybir.dt.float16
AL = mybir.AluOpType
AF = mybir.ActivationFunctionType

B, S, H = 2, 2048, 2048
NH, NKV, HD = 16, 4, 128
FF = 5632
EPS = 1e-5
TPN = 4
QH = NH // TPN           # 4 q heads per core
FFS = FF // TPN          # 1408
FCT = FFS // 128         # 11 FF col tiles
SSL = S // TPN           # 512 output seq cols per core
NHT = H // 128           # 16 H tiles
NST = S // 128           # 16 seq tiles
NSB = S // 512           # 4 seq blocks
GROUPS = [[0, 1, 2, 3], [4, 5, 6, 7]]
PAIRS = [[0, 4], [1, 5], [2, 6], [3, 7]]
SCALE = 1.0 / float(np.sqrt(np.float32(HD)))

# HD permutation: quadrant q: [evens 16q..16q+15 | odds 16q..16q+15]
PERM = np.zeros(HD, dtype=np.int64)
for _q in range(4):
    for _i in range(16):
        PERM[32 * _q + _i] = 2 * (16 * _q + _i)
        PERM[32 * _q + 16 + _i] = 2 * (16 * _q + _i) + 1
SHUF = [(i + 16) % 32 for i in range(32)]


def _sb(x, sb):
    return x[:, sb * 512:(sb + 1) * 512]


def build():
    nc = bacc.Bacc("TRN2", target_bir_lowering=False, debug=False,
                   num_devices=8)

    def din(name, shape, dt=f16):
        return nc.dram_tensor(name, list(shape), dt, kind="ExternalInput")

    xs = din("xs", [512, S])               # H-shard of x[dp].T
    wqh = din("wqh", [1024, TPN * HD])     # permuted cols, row half
    wkh = din("wkh", [1024, HD])           # permuted cols, row half
    wvh = din("wvh", [1024, HD])
    woh = din("woh", [256, H])
    wgh = din("wgh", [1024, FFS])
    wuh = din("wuh", [1024, FFS])
    wdh = din("wdh", [704, H])
    cosT = din("cosT", [64, S], f32)       # cos(ang).T
    sinT = din("sinT", [64, S], f32)
    wn1 = din("wn1", [128, NHT], f32)      # w_norm1[ht*128+p] at [p, ht]
    wn2 = din("wn2", [128, NHT], f32)
    tri = din("tri", [128, 128], f32r)     # tri[k,i] = (i >= k)
    ones = din("ones", [128, 1], f32r)
    epsb = din("epsb", [128, 1], f32)      # EPS bias tile
    ident = din("ident", [128, 128], f32)  # f32 identity
    outsl = nc.dram_tensor("outsl", [H, SSL], f16, kind="ExternalOutput")

    with tile.TileContext(nc) as tc:
        with tc.tile_pool(name="pconst", bufs=1) as pconst, \
             tc.tile_pool(name="pdram", bufs=1, space="DRAM") as pdram:
            ones_t = pconst.tile([128, 1], f32r)
            tri_t = pconst.tile([128, 128], f32r)
            id_t = pconst.tile([128, 128], f32)
            wn1_t = pconst.tile([128, NHT], f32)
            wn2_t = pconst.tile([128, NHT], f32)
            eps_t = pconst.tile([128, 1], f32)
            ctab = pconst.tile([128, S], f32)
            stab = pconst.tile([128, S], f32)
            sT = pconst.tile([64, S], f32)
            nc.sync.dma_start(ones_t[:], ones[:])
            nc.sync.dma_start(tri_t[:], tri[:])
            nc.sync.dma_start(id_t[:], ident[:])
            nc.sync.dma_start(wn1_t[:], wn1[:])
            nc.sync.dma_start(wn2_t[:], wn2[:])
            nc.sync.dma_start(eps_t[:], epsb[:])
            nc.sync.dma_start(sT[:], sinT[:])
            # rope tables: ctab[32q+i] = ctab[32q+16+i] = cos[:, 16q+i]
            #              stab[32q+i] = -sin, stab[32q+16+i] = +sin
            # (engine ops need 32-aligned partition bases; negate once at
            #  partition 0 and bounce via DRAM, then DMA rows into place)
            snegs = pconst.tile([64, S], f32)
            nc.scalar.activation(snegs[:], sT[:], AF.Copy, scale=-1.0)
            for q in range(4):
                nc.sync.dma_start(ctab[32 * q:32 * q + 16, :],
                                  cosT[16 * q:16 * q + 16, :])
                nc.sync.dma_start(ctab[32 * q + 16:32 * q + 32, :],
                                  cosT[16 * q:16 * q + 16, :])
                nc.sync.dma_start(stab[32 * q + 16:32 * q + 32, :],
                                  sinT[16 * q:16 * q + 16, :])

            # DRAM scratch
            snegd = pdram.tile([64, S], f32)
            nc.sync.dma_start(snegd[:], snegs[:])
            for q in range(4):
                nc.sync.dma_start(stab[32 * q:32 * q + 16, :],
                                  snegd[16 * q:16 * q + 16, :])
            xg = pdram.tile([H, S], f16)
            wqf = pdram.tile([H, TPN * HD], f16)
            wkf = pdram.tile([H, HD], f16)
            wvf = pdram.tile([H, HD], f16)
            wof = pdram.tile([QH * HD, H], f16)
            wgf = pdram.tile([H, FFS], f16)
            wuf = pdram.tile([H, FFS], f16)
            wdf = pdram.tile([FFS, H], f16)
            outd = pdram.tile([QH, 128, S], f16)
            ar_in = [pdram.tile([H, 512], f32, name=f"ar_in{i}")
                     for i in range(NSB)]
            ar_out = [pdram.tile([H, 512], f32, name=f"ar_out{i}")
                      for i in range(NSB)]
            mTd = pdram.tile([FCT, 128, S], f16)
            rs_in = pdram.tile([2, NSB, 1024, 512], f16)  # [hh, sb, r, c]
            rs_out = pdram.tile([H, 512], f16)

            # ---------- Phase 0: materialize full x / weights on device ----
            # (collectives cannot read IO tensors; stage via internal DRAM)
            xs_st = pdram.tile([512, S], f16)
            nc.sync.dma_start(xs_st[:], xs[:])
            nc.gpsimd.collective_compute(
                "AllGather", AL.bypass, replica_groups=GROUPS,
                ins=[xs_st[:].opt()], outs=[xg[:].opt()])
            for (src, dst) in ((wkh, wkf), (wvh, wvf), (wqh, wqf),
                               (woh, wof), (wgh, wgf), (wuh, wuf),
                               (wdh, wdf)):
                st = pdram.tile(list(src.shape), f16,
                                name=f"st_{src.name}")
                nc.sync.dma_start(st[:], src[:])
                nc.gpsimd.collective_compute(
                    "AllGather", AL.bypass, replica_groups=PAIRS,
                    ins=[st[:].opt()], outs=[dst[:].opt()])

            with tc.tile_pool(name="phT", bufs=1) as phT:
                hT = phT.tile([128, NHT, S], f16)

                # ---------- Phase A: rmsnorm1 -> hT ----------
                with tc.tile_pool(name="pA", bufs=1) as pA, \
                     tc.tile_pool(name="pAs", bufs=2) as pAs, \
                     tc.tile_pool(name="pAp", bufs=2, space="PSUM") as pAp:
                    for sb in range(NSB):
                        xsb = pA.tile([128, NHT, 512], f16, tag="xsb")
                        ss_ps = pAp.tile([1, 512], f32, tag="ss")
                        for ht in range(NHT):
                            nc.sync.dma_start(
                                xsb[:, ht, :],
                                _sb(xg[ht * 128:(ht + 1) * 128, :], sb))
                            sq = pAs.tile([128, 512], f32r, tag="sq")
                            nc.scalar.activation(sq[:], xsb[:, ht, :],
                                                 AF.Square)
                            nc.tensor.matmul(ss_ps[:], ones_t[:], sq[:],
                                             start=(ht == 0),
                                             stop=(ht == NHT - 1))
                        sd = pAs.tile([1, 512], f32, tag="sd")
                        nc.scalar.activation(sd[:], ss_ps[:], AF.Sqrt,
                                             bias=eps_t[0:1, :],
                                             scale=1.0 / H)
                        rr = pAs.tile([1, 512], f32, tag="rr")
                        nc.vector.reciprocal(rr[:], sd[:])
                        rb = pAs.tile([128, 512], f32, tag="rb")
                        nc.gpsimd.partition_broadcast(rb[:], rr[:])
                        for ht in range(NHT):
                            nc.vector.scalar_tensor_tensor(
                                out=_sb(hT[:, ht, :], sb),
                                in0=xsb[:, ht, :],
                                scalar=wn1_t[:, ht:ht + 1],
                                in1=rb[:], op0=AL.mult, op1=AL.mult)

                # ---------- Phase B: K/V projections + K rope ----------
                with tc.tile_pool(name="pkv", bufs=1) as pkv:
                    kT = pkv.tile([128, S], f32r)
                    v_nat = pkv.tile([128, NST, HD], f32r)

                    with tc.tile_pool(name="pB", bufs=1) as pB, \
                         tc.tile_pool(name="pBw", bufs=1) as pBw, \
                         tc.tile_pool(name="pBp", bufs=2,
                                      space="PSUM") as pBp:
                        wkt = pBw.tile([128, NHT, 128], f16, tag="wB")
                        nc.sync.dma_start(
                            wkt[:],
                            wkf[:].rearrange("(o p) n -> p o n", p=128))
                        for sb in range(NSB):
                            ps = pBp.tile([128, 512], f32, tag="proj")
                            for ht in range(NHT):
                                nc.tensor.matmul(
                                    ps[:], wkt[:, ht, :],
                                    _sb(hT[:, ht, :], sb),
                                    start=(ht == 0), stop=(ht == NHT - 1))
                            qs = pB.tile([128, 512], f32, tag="qs")
                            nc.scalar.copy(qs[:], ps[:])
                            qsw = pB.tile([128, 512], f32, tag="qsw")
                            nc.vector.stream_shuffle(qsw[:], qs[:], SHUF)
                            m2 = pB.tile([128, 512], f32, tag="m2")
                            nc.gpsimd.tensor_mul(m2[:], qsw[:],
                                                 _sb(stab, sb))
                            qc = pB.tile([128, 512], f32, tag="qc")
                            nc.vector.tensor_mul(qc[:], ps[:],
                                                 _sb(ctab, sb))
                            nc.vector.tensor_add(_sb(kT, sb), qc[:], m2[:])
                        # V projection + transpose to natural layout
                        wvt = pBw.tile([128, NHT, 128], f16, tag="wB")
                        nc.sync.dma_start(
                            wvt[:],
                            wvf[:].rearrange("(o p) n -> p o n", p=128))
                        for sb in range(NSB):
                            ps = pBp.tile([128, 512], f32, tag="proj")
                            for ht in range(NHT):
                                nc.tensor.matmul(
                                    ps[:], wvt[:, ht, :],
                                    _sb(hT[:, ht, :], sb),
                                    start=(ht == 0), stop=(ht == NHT - 1))
                            vts = pB.tile([128, 512], f32, tag="vts")
                            nc.scalar.copy(vts[:], ps[:])
                            for k4 in range(4):
                                pt = pBp.tile([128, 128], f32, tag="vtr")
                                nc.tensor.transpose(
                                    pt[:], vts[:, k4 * 128:(k4 + 1) * 128],
                                    id_t[:])
                                nc.scalar.copy(v_nat[:, sb * 4 + k4, :],
                                               pt[:])

                    # ------- Phase C: per-head Q proj + rope + attention ----
                    with tc.tile_pool(name="pq", bufs=1) as pq, \
                         tc.tile_pool(name="pC", bufs=2) as pC, \
                         tc.tile_pool(name="pCw", bufs=1) as pCw, \
                         tc.tile_pool(name="pCp", bufs=2,
                                      space="PSUM") as pCp, \
                         tc.tile_pool(name="pCo", bufs=1,
                                      space="PSUM") as pCo:
                        for h in range(QH):
                            qTh = pq.tile([128, S], f32r, tag="qTh")
                            wqt = pCw.tile([128, NHT, 128], f16, tag="wq")
                            nc.sync.dma_start(
                                wqt[:],
                                wqf[:].rearrange("(o p) n -> p o n", p=128)
                                   [:, :, h * 128:(h + 1) * 128])
                            for sb in range(NSB):
                                ps = pCp.tile([128, 512], f32, tag="proj2")
                                for ht in range(NHT):
                                    nc.tensor.matmul(
                                        ps[:], wqt[:, ht, :],
                                        _sb(hT[:, ht, :], sb),
                                        start=(ht == 0),
                                        stop=(ht == NHT - 1))
                                qs = pC.tile([128, 512], f32, tag="qs2",
                                             bufs=1)
                                nc.scalar.copy(qs[:], ps[:])
                                qsw = pC.tile([128, 512], f32, tag="qsw2",
                                              bufs=1)
                                nc.vector.stream_shuffle(qsw[:], qs[:],
                                                         SHUF)
                                m2 = pC.tile([128, 512], f32, tag="m22",
                                             bufs=1)
                                nc.gpsimd.tensor_mul(m2[:], qsw[:],
                                                     _sb(stab, sb))
                                qc = pC.tile([128, 512], f32, tag="qc2",
                                             bufs=1)
                                nc.vector.tensor_mul(qc[:], ps[:],
                                                     _sb(ctab, sb))
                                nc.vector.tensor_add(_sb(qTh, sb),
                                                     qc[:], m2[:])
                            # attention for this head
                            for qb in range(NSB):
                                acc = pCo.tile([128, 512], f32, tag="acc")
                                den = pCo.tile([1, 512], f32, tag="den")
                                nkt = 4 * (qb + 1)
                                for kt in range(nkt):
                                    j = kt - qb * 4
                                    coloff = max(0, j) * 128
                                    ncols = 512 - coloff
                                    qs0 = qb * 512 + coloff
                                    sc = pCp.tile([128, 512], f32,
                                                  tag="sc")
                                    nc.tensor.matmul(
                                        sc[:, 0:ncols],
                                        kT[:, kt * 128:(kt + 1) * 128],
                                        qTh[:, qs0:qs0 + ncols],
                                        start=True, stop=True)
                                    P = pC.tile([128, 512], f32r,
                                                tag="P", bufs=3)
                                    nc.scalar.activation(
                                        P[:, 0:ncols], sc[:, 0:ncols],
                                        AF.Exp, scale=SCALE)
                                    if j >= 0:
                                        nc.vector.tensor_mul(
                                            P[:, 0:128], P[:, 0:128],
                                            tri_t[:])
                                    nc.tensor.matmul(
                                        acc[:, coloff:512],
                                        v_nat[:, kt, :], P[:, 0:ncols],
                                        start=(kt == 0),
                                        stop=(kt == nkt - 1))
                                    nc.tensor.matmul(
                                        den[0:1, coloff:512], ones_t[:],
                                        P[:, 0:ncols],
                                        start=(kt == 0),
                                        stop=(kt == nkt - 1))
                                rd = pC.tile([1, 512], f32, tag="rd")
                                nc.vector.reciprocal(rd[:], den[:])
                                rb = pC.tile([128, 512], f32, tag="rb2")
                                nc.gpsimd.partition_broadcast(rb[:], rd[:])
                                ot = pC.tile([128, 512], f16, tag="ot")
                                nc.vector.tensor_mul(ot[:], acc[:], rb[:])
                                nc.sync.dma_start(
                                    _sb(outd[h, :, :], qb), ot[:])

                    # ---- Phase D: Wo partial + chunked AllReduce ----
                    with tc.tile_pool(name="pD", bufs=2) as pD, \
                         tc.tile_pool(name="pDw", bufs=1) as pDw, \
                         tc.tile_pool(name="pDp", bufs=2,
                                      space="PSUM") as pDp:
                        wo_t = pDw.tile([128, QH, NHT, 128], f16)
                        for k2 in range(QH):
                            nc.sync.dma_start(
                                wo_t[:, k2, :, :].rearrange(
                                    "p a b -> p (a b)"),
                                wof[k2 * 128:(k2 + 1) * 128, :])
                        for sb in range(NSB):
                            osb = pD.tile([128, QH, 512], f16,
                                          tag="osb", bufs=1)
                            nc.sync.dma_start(
                                osb[:],
                                outd[:, :, sb * 512:(sb + 1) * 512]
                                .rearrange("o p n -> p o n"))
                            for ocg in range(2):
                                xqg = pD.tile([128, 8, 512], f16,
                                              tag="xqg", bufs=1)
                                nc.sync.dma_start(
                                    xqg[:],
                                    xg[:].rearrange("(a p) n -> p a n",
                                                    p=128)
                                    [:, ocg * 8:(ocg + 1) * 8,
                                     sb * 512:(sb + 1) * 512])
                                for oc8 in range(8):
                                    oc = ocg * 8 + oc8
                                    ps = pDp.tile([128, 512], f32,
                                                  tag="y")
                                    for k2 in range(QH):
                                        nc.tensor.matmul(
                                            ps[:],
                                            wo_t[:, k2, oc, :],
                                            osb[:, k2, :],
                                            start=(k2 == 0),
                                            stop=(k2 == QH - 1))
                                    yt = pD.tile([128, 512], f32,
                                                 tag="yt")
                                    nc.vector.scalar_tensor_tensor(
                                        out=yt[:], in0=xqg[:, oc8, :],
                                        scalar=0.25, in1=ps[:],
                                        op0=AL.mult, op1=AL.add)
                                    nc.sync.dma_start(
                                        ar_in[sb][oc * 128:
                                                  (oc + 1) * 128, :],
                                        yt[:])
                            nc.gpsimd.collective_compute(
                                "AllReduce", AL.add,
                                replica_groups=GROUPS,
                                ins=[ar_in[sb].opt()],
                                outs=[ar_out[sb].opt()])

            # ---------- Phase E: x1 = xT + ar; rmsnorm2 -> h2T ----------
            with tc.tile_pool(name="ph2", bufs=1) as ph2:
                h2T = ph2.tile([128, NHT, S], f16)
                with tc.tile_pool(name="pE", bufs=1) as pE, \
                     tc.tile_pool(name="pEs", bufs=2) as pEs, \
                     tc.tile_pool(name="pEp", bufs=2, space="PSUM") as pEp:
                    for sb in range(NSB):
                        x1sb = pE.tile([128, NHT, 512], f32, tag="x1sb")
                        ss_ps = pEp.tile([1, 512], f32, tag="ss2")
                        for ht in range(NHT):
                            nc.sync.dma_start(
                                x1sb[:, ht, :],
                                ar_out[sb][ht * 128:(ht + 1) * 128, :])
                            sq = pEs.tile([128, 512], f32r, tag="sq2")
                            nc.scalar.activation(sq[:], x1sb[:, ht, :],
                                                 AF.Square)
                            nc.tensor.matmul(ss_ps[:], ones_t[:], sq[:],
                                             start=(ht == 0),
                                             stop=(ht == NHT - 1))
                        sd = pEs.tile([1, 512], f32, tag="sd2")
                        nc.scalar.activation(sd[:], ss_ps[:], AF.Sqrt,
                                             bias=eps_t[0:1, :],
                                             scale=1.0 / H)
                        rr = pEs.tile([1, 512], f32, tag="rr2")
                        nc.vector.reciprocal(rr[:], sd[:])
                        rb = pEs.tile([128, 512], f32, tag="rb3")
                        nc.gpsimd.partition_broadcast(rb[:], rr[:])
                        for ht in range(NHT):
                            nc.vector.scalar_tensor_tensor(
                                out=_sb(h2T[:, ht, :], sb),
                                in0=x1sb[:, ht, :],
                                scalar=wn2_t[:, ht:ht + 1],
                                in1=rb[:], op0=AL.mult, op1=AL.mult)

                # ---------- Phase F1: gate/up/silu-mul -> mT (DRAM) -------
                with tc.tile_pool(name="pF", bufs=2) as pF, \
                     tc.tile_pool(name="pFw", bufs=2) as pFw, \
                     tc.tile_pool(name="pFp", bufs=2, space="PSUM") as pFp:
                    for ct in range(FCT):
                        wgt = pFw.tile([128, NHT, 128], f16, tag="wg")
                        wut = pFw.tile([128, NHT, 128], f16, tag="wu")
                        nc.sync.dma_start(
                            wgt[:],
                            wgf[:].rearrange("(o p) n -> p o n", p=128)
                               [:, :, ct * 128:(ct + 1) * 128])
                        nc.sync.dma_start(
                            wut[:],
                            wuf[:].rearrange("(o p) n -> p o n", p=128)
                               [:, :, ct * 128:(ct + 1) * 128])
                        for sb in range(NSB):
                            pg = pFp.tile([128, 512], f32, tag="pg")
                            pu = pFp.tile([128, 512], f32, tag="pu")
                            for ht in range(NHT):
                                nc.tensor.matmul(
                                    pg[:], wgt[:, ht, :],
                                    _sb(h2T[:, ht, :], sb),
                                    start=(ht == 0), stop=(ht == NHT - 1))
                            for ht in range(NHT):
                                nc.tensor.matmul(
                                    pu[:], wut[:, ht, :],
                                    _sb(h2T[:, ht, :], sb),
                                    start=(ht == 0), stop=(ht == NHT - 1))
                            sg = pF.tile([128, 512], f32, tag="sg")
                            nc.scalar.activation(sg[:], pg[:], AF.Silu)
                            mt = pF.tile([128, 512], f16, tag="mt")
                            nc.vector.tensor_mul(mt[:], pu[:], sg[:])
                            nc.sync.dma_start(
                                _sb(mTd[ct, :, :], sb), mt[:])

            # ---------- Phase F2: down + 0.25*x1 -> chunked RS --------
            with tc.tile_pool(name="pwd", bufs=1) as pwd, \
                 tc.tile_pool(name="pGm", bufs=1) as pGm, \
                 tc.tile_pool(name="pG", bufs=2) as pG, \
                 tc.tile_pool(name="pGp", bufs=2, space="PSUM") as pGp:
                mm = pGm.tile([128, FCT, S], f16)
                for ct in range(FCT):
                    nc.sync.dma_start(mm[:, ct, :], mTd[ct, :, :])
                for oc in range(NHT):
                    wdo = pwd.tile([128, FCT, 128], f16, tag="wdo",
                                   bufs=2)
                    nc.sync.dma_start(
                        wdo[:],
                        wdf[:].rearrange("(a p) n -> p a n", p=128)
                        [:, :, oc * 128:(oc + 1) * 128])
                    for sb in range(NSB):
                        ps = pGp.tile([128, 512], f32, tag="pd")
                        for ct in range(FCT):
                            nc.tensor.matmul(
                                ps[:], wdo[:, ct, :],
                                mm[:, ct, sb * 512:(sb + 1) * 512],
                                start=(ct == 0), stop=(ct == FCT - 1))
                        x1t = pG.tile([128, 512], f32, tag="x1t")
                        nc.sync.dma_start(
                            x1t[:],
                            ar_out[sb][oc * 128:(oc + 1) * 128, :])
                        yd = pG.tile([128, 512], f16, tag="yd")
                        nc.vector.scalar_tensor_tensor(
                            out=yd[:], in0=x1t[:], scalar=0.25,
                            in1=ps[:], op0=AL.mult, op1=AL.add)
                        nc.sync.dma_start(
                            rs_in[oc // 8, sb,
                                  (oc % 8) * 128:(oc % 8 + 1) * 128, :],
                            yd[:])
                    if oc % 8 == 7:
                        hh = oc // 8
                        nc.gpsimd.collective_compute(
                            "ReduceScatter", AL.add, replica_groups=GROUPS,
                            ins=[rs_in[hh].opt()],
                            outs=[rs_out[hh * 1024:(hh + 1) * 1024, :]
                                  .opt()])

            # ---------- Phase G: write output ----------
            nc.sync.dma_start(outsl[:], rs_out[:])

    nc.finalize()
    return nc


_CACHE = {}


def _get_nc():
    if "nc" not in _CACHE:
        _CACHE["nc"] = build()
    return _CACHE["nc"]


# Inputs that depend only on the weights/rope/norm tensors (not on x).
# These stay device-resident across calls; a full content-equality check
# on the raw inputs guards correctness for arbitrary inputs.
_STATIC_NAMES = ("wqh", "wkh", "wvh", "woh", "wgh", "wuh", "wdh",
                 "cosT", "sinT", "wn1", "wn2", "tri", "ones", "epsb",
                 "ident")
_STATIC_RAW_KEYS = ("Wq", "Wk", "Wv", "Wo", "Wgate", "Wup", "Wdown",
                    "w_norm1", "w_norm2", "freqs_cos", "freqs_sin")


def _get_runner():
    """Build the jitted shard_map executable ONCE; reuse across calls."""
    if "runner" in _CACHE:
        return _CACHE["runner"]

    import jax
    import jax.numpy as jnp
    from jax.sharding import Mesh, PartitionSpec, NamedSharding
    from jax.experimental.shard_map import shard_map
    from concourse import bass2jax

    bass2jax.install_neuronx_cc_hook()
    nc = _get_nc()
    n_cores = 8

    partition_name = (nc.partition_id_tensor.name
                      if nc.partition_id_tensor else None)
    in_names, out_names, out_avals, zero_outs = [], [], [], []
    for alloc in nc.m.functions[0].allocations:
        if not isinstance(alloc, mybir.MemoryLocationSet):
            continue
        name = alloc.memorylocations[0].name
        if alloc.kind == "ExternalInput":
            if name != partition_name:
                in_names.append(name)
        elif alloc.kind == "ExternalOutput":
            shape = tuple(alloc.tensor_shape)
            dtype = mybir.dt.np(alloc.dtype)
            out_names.append(name)
            out_avals.append(jax.core.ShapedArray(shape, dtype))
            zero_outs.append(np.zeros(shape, dtype))
    n_params = len(in_names)
    n_outs = len(out_avals)
    all_names = list(in_names) + list(out_names)
    if partition_name is not None:
        all_names.append(partition_name)
    donate = tuple(range(n_params, n_params + n_outs))

    def _body(*args):
        operands = list(args)
        if partition_name is not None:
            operands.append(bass2jax.partition_id_tensor())
        outs = bass2jax._bass_exec_p.bind(
            *operands,
            out_avals=tuple(out_avals),
            in_names=tuple(all_names),
            out_names=tuple(out_names),
            lowering_input_output_aliases=(),
            sim_require_finite=True,
            sim_require_nnan=True,
            nc=nc,
        )
        return tuple(outs)

    devices = jax.devices()[:n_cores]
    mesh = Mesh(np.asarray(devices), ("core",))
    sh = NamedSharding(mesh, PartitionSpec("core"))
    in_specs = (PartitionSpec("core"),) * (n_params + n_outs)
    out_specs = (PartitionSpec("core"),) * n_outs
    sharded = jax.jit(
        shard_map(_body, mesh=mesh, in_specs=in_specs,
                  out_specs=out_specs, check_rep=False),
        donate_argnums=donate,
        keep_unused=True,
    )

    # donated zero output buffers, generated on device (no host upload)
    zero_shapes = [(n_cores * z.shape[0], *z.shape[1:]) for z in zero_outs]
    zero_dtypes = [z.dtype for z in zero_outs]
    make_zeros = jax.jit(
        lambda: tuple(jnp.zeros(s, d)
                      for s, d in zip(zero_shapes, zero_dtypes)),
        out_shardings=tuple(sh for _ in zero_shapes),
    )

    def run(in_maps, static_dev):
        """static_dev: dict name -> device array (sharded) or None."""
        args = []
        for name in in_names:
            if static_dev is not None and name in static_dev:
                args.append(static_dev[name])
            else:
                args.append(np.concatenate(
                    [np.asarray(in_maps[c][name]) for c in range(n_cores)],
                    axis=0))
        out_arrs = sharded(*args, *make_zeros())
        return [
            {name: np.asarray(out_arrs[i]).reshape(
                n_cores, *out_avals[i].shape)[c]
             for i, name in enumerate(out_names)}
            for c in range(n_cores)
        ]

    def put_static(in_maps):
        """Upload the static inputs once; returns dict of device arrays."""
        dev = {}
        for name in _STATIC_NAMES:
            glob = np.concatenate(
                [np.asarray(in_maps[c][name]) for c in range(n_cores)],
                axis=0)
            dev[name] = jax.device_put(glob, sh)
        for a in dev.values():
            a.block_until_ready()
        return dev

    _CACHE["runner"] = (run, put_static)
    return _CACHE["runner"]


def _host_prep_static(inputs):
    """Per-core maps for the weight-derived (x-independent) inputs."""
    Wq = np.asarray(inputs["Wq"], np.float32).astype(np.float16)
    Wk = np.asarray(inputs["Wk"], np.float32).astype(np.float16)
    Wv = np.asarray(inputs["Wv"], np.float32).astype(np.float16)
    Wo = np.asarray(inputs["Wo"], np.float32).astype(np.float16)
    Wg = np.asarray(inputs["Wgate"], np.float32).astype(np.float16)
    Wu = np.asarray(inputs["Wup"], np.float32).astype(np.float16)
    Wd = np.asarray(inputs["Wdown"], np.float32).astype(np.float16)
    wn1v = np.asarray(inputs["w_norm1"], np.float32)
    wn2v = np.asarray(inputs["w_norm2"], np.float32)
    cos = np.asarray(inputs["freqs_cos"], np.float32)
    sin = np.asarray(inputs["freqs_sin"], np.float32)

    tri_np = (np.arange(128)[None, :] >= np.arange(128)[:, None])
    tri_np = tri_np.astype(np.float32)
    wn1_np = np.ascontiguousarray(wn1v.reshape(NHT, 128).T)
    wn2_np = np.ascontiguousarray(wn2v.reshape(NHT, 128).T)

    shared = dict(cosT=np.ascontiguousarray(cos.T),
                  sinT=np.ascontiguousarray(sin.T),
                  wn1=wn1_np, wn2=wn2_np, tri=tri_np,
                  ones=np.ones((128, 1), np.float32),
                  ident=np.eye(128, dtype=np.float32),
                  epsb=np.full((128, 1), EPS, np.float32))

    halves = []      # halves[dp][tp] -> dict of weight-half arrays
    for dp in range(2):
        r0, r1 = dp * 1024, (dp + 1) * 1024
        per_tp = []
        for tp in range(TPN):
            qcols = []
            for h in range(tp * QH, (tp + 1) * QH):
                qcols.extend(h * HD + PERM)
            per_tp.append(dict(
                wqh=Wq[r0:r1][:, qcols],
                wkh=Wk[r0:r1][:, tp * HD + PERM],
                wvh=np.ascontiguousarray(
                    Wv[r0:r1, tp * HD:(tp + 1) * HD]),
                woh=np.ascontiguousarray(
                    Wo[tp * QH * HD + dp * 256:
                       tp * QH * HD + (dp + 1) * 256, :]),
                wgh=np.ascontiguousarray(
                    Wg[r0:r1, tp * FFS:(tp + 1) * FFS]),
                wuh=np.ascontiguousarray(
                    Wu[r0:r1, tp * FFS:(tp + 1) * FFS]),
                wdh=np.ascontiguousarray(
                    Wd[tp * FFS + dp * 704:tp * FFS + (dp + 1) * 704, :]),
            ))
        halves.append(per_tp)

    in_maps = []
    for c in range(8):
        dp, tp = c // 4, c % 4
        m = dict(shared)
        m.update(halves[dp][tp])
        in_maps.append(m)
    return in_maps


def _host_prep_dynamic(inputs):
    """Per-core maps for the x-derived inputs."""
    x = np.asarray(inputs["x"], np.float32)
    in_maps = []
    for c in range(8):
        dp, tp = c // 4, c % 4
        in_maps.append(dict(xs=np.ascontiguousarray(
            x[dp][:, tp * 512:(tp + 1) * 512].T).astype(np.float16)))
    return in_maps


def _statics_unchanged(inputs):
    cached = _CACHE.get("static_raw")
    if cached is None:
        return False
    for k in _STATIC_RAW_KEYS:
        a, b = cached[k], inputs[k]
        if a is b:
            continue
        a = np.asarray(a)
        b = np.asarray(b)
        if a.shape != b.shape or a.dtype != b.dtype or \
                not np.array_equal(a, b):
            return False
    return True


def kernel(**inputs) -> np.ndarray:
    run, put_static = _get_runner()
    if not _statics_unchanged(inputs):
        smaps = _host_prep_static(inputs)
        _CACHE["static_dev"] = put_static(smaps)
        _CACHE["static_raw"] = {k: inputs[k] for k in _STATIC_RAW_KEYS}
    dmaps = _host_prep_dynamic(inputs)
    res = run(dmaps, _CACHE["static_dev"])
    out = np.zeros((B, S, H), np.float32)
    for c in range(8):
        dp, tp = c // 4, c % 4
        sl = res[c]["outsl"]                  # [H, 512] fp16
        out[dp, tp * SSL:(tp + 1) * SSL, :] = sl.T.astype(np.float32)
    return out


# revision 16
# speedup vs baseline: 32.0002x; 1.1984x over previous
"""Trainium2 Bass kernel for nn_MiniAgentBlock (dense transformer block).

Sharding: DP=2 over batch x TP=4 within each batch (8 NeuronCores).
Core c: dp = c//4 (batch), tp = c%4 (4 q-heads / 1 kv-head, FF/4 slice).

Wall-clock optimizations over the first working version:
- The jitted shard_map executable is built ONCE and cached; repeat calls
  skip jax re-trace / XLA+neuronxcc re-compile / NEFF reload.
- All large inputs ship as fp16 (error budget: rel tol 2e-2, fp16
  quantization contributes ~1e-3).
- No duplicated bytes over the (slow, ~40MB/s) axon tunnel:
  x ships as per-core [512, S] H-shards, AllGathered on device across
  the TP group; every weight ships as a half split along its input dim
  across the DP pair (cores c, c+4 hold the same TP slice), AllGathered
  on device across pair groups [[0,4],[1,5],[2,6],[3,7]].
- Rope tables ship as compact [64, S] cos/sin, expanded on device into
  SBUF; the 1/sqrt(HD) score scale is folded into the Exp activation.
- Output returns as fp16 [H, S/4] per core.

Device kernel: all matmul phases run in transposed [feature, seq]
layout; projection/FFN matmuls in fp16 (full PE rate), attention in
fp32r. On-device AllReduce after the attention output projection and
ReduceScatter after the FFN down projection, within each 4-core group.
The residual x1 = x + attn is folded into the ReduceScatter as 0.25*x1
per core, so the program is identical on every core (pure SPMD).
"""
import sys
if "/opt/trn_rl_repo" not in sys.path:
    sys.path.insert(0, "/opt/trn_rl_repo")

import numpy as np
import concourse.bass as bass
import concourse.mybir as mybir
import concourse.tile as tile
from concourse import bacc

f32 = mybir.dt.float32
f32r = mybir.dt.float32r
f16 = mybir.dt.float16
AL = mybir.AluOpType
AF = mybir.ActivationFunctionType

B, S, H = 2, 2048, 2048
NH, NKV, HD = 16, 4, 128
FF = 5632
EPS = 1e-5
TPN = 4
QH = NH // TPN           # 4 q heads per core
FFS = FF // TPN          # 1408
FCT = FFS // 128         # 11 FF col tiles
SSL = S // TPN           # 512 output seq cols per core
NHT = H // 128           # 16 H tiles
NST = S // 128           # 16 seq tiles
NSB = S // 512           # 4 seq blocks
GROUPS = [[0, 1, 2, 3], [4, 5, 6, 7]]
PAIRS = [[0, 4], [1, 5], [2, 6], [3, 7]]
SCALE = 1.0 / float(np.sqrt(np.float32(HD)))

# HD permutation: quadrant q: [evens 16q..16q+15 | odds 16q..16q+15]
PERM = np.zeros(HD, dtype=np.int64)
for _q in range(4):
    for _i in range(16):
        PERM[32 * _q + _i] = 2 * (16 * _q + _i)
        PERM[32 * _q + 16 + _i] = 2 * (16 * _q + _i) + 1
SHUF = [(i + 16) % 32 for i in range(32)]


def _sb(x, sb):
    return x[:, sb * 512:(sb + 1) * 512]


def build():
    nc = bacc.Bacc("TRN2", target_bir_lowering=False, debug=False,
                   num_devices=8)

    def din(name, shape, dt=f16):
        return nc.dram_tensor(name, list(shape), dt, kind="ExternalInput")

    xs = din("xs", [512, S])               # H-shard of x[dp].T
    wqh = din("wqh", [1024, TPN * HD])     # permuted cols, row half
    wkh = din("wkh", [1024, HD])           # permuted cols, row half
    wvh = din("wvh", [1024, HD])
    woh = din("woh", [256, H])
    wgh = din("wgh", [1024, FFS])
    wuh = din("wuh", [1024, FFS])
    wdh = din("wdh", [704, H])
    cosT = din("cosT", [64, S], f32)       # cos(ang).T
    sinT = din("sinT", [64, S], f32)
    wn1 = din("wn1", [128, NHT], f32)      # w_norm1[ht*128+p] at [p, ht]
    wn2 = din("wn2", [128, NHT], f32)
    tri = din("tri", [128, 128], f32r)     # tri[k,i] = (i >= k)
    ones = din("ones", [128, 1], f32r)
    epsb = din("epsb", [128, 1], f32)      # EPS bias tile
    ident = din("ident", [128, 128], f32)  # f32 identity
    outsl = nc.dram_tensor("outsl", [H, SSL], f16, kind="ExternalOutput")

    with tile.TileContext(nc) as tc:
        with tc.tile_pool(name="pconst", bufs=1) as pconst, \
             tc.tile_pool(name="pdram", bufs=1, space="DRAM") as pdram:
            ones_t = pconst.tile([128, 1], f32r)
            tri_t = pconst.tile([128, 128], f32r)
            id_t = pconst.tile([128, 128], f32)
            wn1_t = pconst.tile([128, NHT], f32)
            wn2_t = pconst.tile([128, NHT], f32)
            eps_t = pconst.tile([128, 1], f32)
            ctab = pconst.tile([128, S], f32)
            stab = pconst.tile([128, S], f32)
            sT = pconst.tile([64, S], f32)
            nc.sync.dma_start(ones_t[:], ones[:])
            nc.sync.dma_start(tri_t[:], tri[:])
            nc.sync.dma_start(id_t[:], ident[:])
            nc.sync.dma_start(wn1_t[:], wn1[:])
            nc.sync.dma_start(wn2_t[:], wn2[:])
            nc.sync.dma_start(eps_t[:], epsb[:])
            nc.sync.dma_start(sT[:], sinT[:])
            # rope tables: ctab[32q+i] = ctab[32q+16+i] = cos[:, 16q+i]
            #              stab[32q+i] = -sin, stab[32q+16+i] = +sin
            # (engine ops need 32-aligned partition bases; negate once at
            #  partition 0 and bounce via DRAM, then DMA rows into place)
            snegs = pconst.tile([64, S], f32)
            nc.scalar.activation(snegs[:], sT[:], AF.Copy, scale=-1.0)
            for q in range(4):
                nc.sync.dma_start(ctab[32 * q:32 * q + 16, :],
                                  cosT[16 * q:16 * q + 16, :])
                nc.sync.dma_start(ctab[32 * q + 16:32 * q + 32, :],
                                  cosT[16 * q:16 * q + 16, :])
                nc.sync.dma_start(stab[32 * q + 16:32 * q + 32, :],
                                  sinT[16 * q:16 * q + 16, :])

            # DRAM scratch
            snegd = pdram.tile([64, S], f32)
            nc.sync.dma_start(snegd[:], snegs[:])
            for q in range(4):
                nc.sync.dma_start(stab[32 * q:32 * q + 16, :],
                                  snegd[16 * q:16 * q + 16, :])
            xg = pdram.tile([H, S], f16)
            wqf = pdram.tile([H, TPN * HD], f16)
            wkf = pdram.tile([H, HD], f16)
            wvf = pdram.tile([H, HD], f16)
            wof = pdram.tile([QH * HD, H], f16)
            wgf = pdram.tile([H, FFS], f16)
            wuf = pdram.tile([H, FFS], f16)
            wdf = pdram.tile([FFS, H], f16)
            outd = pdram.tile([QH, 128, S], f16)
            ar_in = [pdram.tile([H, 512], f32, name=f"ar_in{i}")
                     for i in range(NSB)]
            ar_out = [pdram.tile([H, 512], f32, name=f"ar_out{i}")
                      for i in range(NSB)]
            mTd = pdram.tile([FCT, 128, S], f16)
            rs_in = pdram.tile([2, NSB, 1024, 512], f16)  # [hh, sb, r, c]
            rs_out = pdram.tile([H, 512], f16)

            # ---------- Phase 0: materialize full x / weights on device ----
            # (collectives cannot read IO tensors; stage via internal DRAM)
            xs_st = pdram.tile([512, S], f16)
            nc.sync.dma_start(xs_st[:], xs[:])
            nc.gpsimd.collective_compute(
                "AllGather", AL.bypass, replica_groups=GROUPS,
                ins=[xs_st[:].opt()], outs=[xg[:].opt()])
            for (src, dst) in ((wkh, wkf), (wvh, wvf), (wqh, wqf),
                               (woh, wof), (wgh, wgf), (wuh, wuf),
                               (wdh, wdf)):
                st = pdram.tile(list(src.shape), f16,
                                name=f"st_{src.name}")
                nc.sync.dma_start(st[:], src[:])
                nc.gpsimd.collective_compute(
                    "AllGather", AL.bypass, replica_groups=PAIRS,
                    ins=[st[:].opt()], outs=[dst[:].opt()])

            with tc.tile_pool(name="phT", bufs=1) as phT:
                hT = phT.tile([128, NHT, S], f16)

                # ---------- Phase A: rmsnorm1 -> hT ----------
                with tc.tile_pool(name="pA", bufs=1) as pA, \
                     tc.tile_pool(name="pAs", bufs=2) as pAs, \
                     tc.tile_pool(name="pAp", bufs=2, space="PSUM") as pAp:
                    for sb in range(NSB):
                        xsb = pA.tile([128, NHT, 512], f16, tag="xsb")
                        ss_ps = pAp.tile([1, 512], f32, tag="ss")
                        for ht in range(NHT):
                            nc.sync.dma_start(
                                xsb[:, ht, :],
                                _sb(xg[ht * 128:(ht + 1) * 128, :], sb))
                            sq = pAs.tile([128, 512], f32r, tag="sq")
                            nc.scalar.activation(sq[:], xsb[:, ht, :],
                                                 AF.Square)
                            nc.tensor.matmul(ss_ps[:], ones_t[:], sq[:],
                                             start=(ht == 0),
                                             stop=(ht == NHT - 1))
                        sd = pAs.tile([1, 512], f32, tag="sd")
                        nc.scalar.activation(sd[:], ss_ps[:], AF.Sqrt,
                                             bias=eps_t[0:1, :],
                                             scale=1.0 / H)
                        rr = pAs.tile([1, 512], f32, tag="rr")
                        nc.vector.reciprocal(rr[:], sd[:])
                        rb = pAs.tile([128, 512], f32, tag="rb")
                        nc.gpsimd.partition_broadcast(rb[:], rr[:])
                        for ht in range(NHT):
                            nc.vector.scalar_tensor_tensor(
                                out=_sb(hT[:, ht, :], sb),
                                in0=xsb[:, ht, :],
                                scalar=wn1_t[:, ht:ht + 1],
                                in1=rb[:], op0=AL.mult, op1=AL.mult)

                # ---------- Phase B: K/V projections + K rope ----------
                with tc.tile_pool(name="pkv", bufs=1) as pkv:
                    kT = pkv.tile([128, S], f32r)
                    v_nat = pkv.tile([128, NST, HD], f32r)

                    with tc.tile_pool(name="pB", bufs=1) as pB, \
                         tc.tile_pool(name="pBw", bufs=1) as pBw, \
                         tc.tile_pool(name="pBp", bufs=2,
                                      space="PSUM") as pBp:
                        wkt = pBw.tile([128, NHT, 128], f16, tag="wB")
                        nc.sync.dma_start(
                            wkt[:],
                            wkf[:].rearrange("(o p) n -> p o n", p=128))
                        for sb in range(NSB):
                            ps = pBp.tile([128, 512], f32, tag="proj")
                            for ht in range(NHT):
                                nc.tensor.matmul(
                                    ps[:], wkt[:, ht, :],
                                    _sb(hT[:, ht, :], sb),
                                    start=(ht == 0), stop=(ht == NHT - 1))
                            qs = pB.tile([128, 512], f32, tag="qs")
                            nc.scalar.copy(qs[:], ps[:])
                            qsw = pB.tile([128, 512], f32, tag="qsw")
                            nc.vector.stream_shuffle(qsw[:], qs[:], SHUF)
                            m2 = pB.tile([128, 512], f32, tag="m2")
                            nc.gpsimd.tensor_mul(m2[:], qsw[:],
                                                 _sb(stab, sb))
                            qc = pB.tile([128, 512], f32, tag="qc")
                            nc.vector.tensor_mul(qc[:], ps[:],
                                                 _sb(ctab, sb))
                            nc.vector.tensor_add(_sb(kT, sb), qc[:], m2[:])
                        # V projection + transpose to natural layout
                        wvt = pBw.tile([128, NHT, 128], f16, tag="wB")
                        nc.sync.dma_start(
                            wvt[:],
                            wvf[:].rearrange("(o p) n -> p o n", p=128))
                        for sb in range(NSB):
                            ps = pBp.tile([128, 512], f32, tag="proj")
                            for ht in range(NHT):
                                nc.tensor.matmul(
                                    ps[:], wvt[:, ht, :],
                                    _sb(hT[:, ht, :], sb),
                                    start=(ht == 0), stop=(ht == NHT - 1))
                            vts = pB.tile([128, 512], f32, tag="vts")
                            nc.scalar.copy(vts[:], ps[:])
                            for k4 in range(4):
                                pt = pBp.tile([128, 128], f32, tag="vtr")
                                nc.tensor.transpose(
                                    pt[:], vts[:, k4 * 128:(k4 + 1) * 128],
                                    id_t[:])
                                nc.scalar.copy(v_nat[:, sb * 4 + k4, :],
                                               pt[:])

                    # ------- Phase C: per-head Q proj + rope + attention ----
                    with tc.tile_pool(name="pq", bufs=1) as pq, \
                         tc.tile_pool(name="pC", bufs=2) as pC, \
                         tc.tile_pool(name="pCw", bufs=1) as pCw, \
                         tc.tile_pool(name="pCp", bufs=2,
                                      space="PSUM") as pCp, \
                         tc.tile_pool(name="pCo", bufs=1,
                                      space="PSUM") as pCo:
                        for h in range(QH):
                            qTh = pq.tile([128, S], f32r, tag="qTh")
                            wqt = pCw.tile([128, NHT, 128], f16, tag="wq")
                            nc.sync.dma_start(
                                wqt[:],
                                wqf[:].rearrange("(o p) n -> p o n", p=128)
                                   [:, :, h * 128:(h + 1) * 128])
                            for sb in range(NSB):
                                ps = pCp.tile([128, 512], f32, tag="proj2")
                                for ht in range(NHT):
                                    nc.tensor.matmul(
                                        ps[:], wqt[:, ht, :],
                                        _sb(hT[:, ht, :], sb),
                                        start=(ht == 0),
                                        stop=(ht == NHT - 1))
                                qs = pC.tile([128, 512], f32, tag="qs2",
                                             bufs=1)
                                nc.scalar.copy(qs[:], ps[:])
                                qsw = pC.tile([128, 512], f32, tag="qsw2",
                                              bufs=1)
                                nc.vector.stream_shuffle(qsw[:], qs[:],
                                                         SHUF)
                                m2 = pC.tile([128, 512], f32, tag="m22",
                                             bufs=1)
                                nc.gpsimd.tensor_mul(m2[:], qsw[:],
                                                     _sb(stab, sb))
                                qc = pC.tile([128, 512], f32, tag="qc2",
                                             bufs=1)
                                nc.vector.tensor_mul(qc[:], ps[:],
                                                     _sb(ctab, sb))
                                nc.vector.tensor_add(_sb(qTh, sb),
                                                     qc[:], m2[:])
                            # attention for this head
                            for qb in range(NSB):
                                acc = pCo.tile([128, 512], f32, tag="acc")
                                den = pCo.tile([1, 512], f32, tag="den")
                                nkt = 4 * (qb + 1)
                                for kt in range(nkt):
                                    j = kt - qb * 4
                                    coloff = max(0, j) * 128
                                    ncols = 512 - coloff
                                    qs0 = qb * 512 + coloff
                                    sc = pCp.tile([128, 512], f32,
                                                  tag="sc")
                                    nc.tensor.matmul(
                                        sc[:, 0:ncols],
                                        kT[:, kt * 128:(kt + 1) * 128],
                                        qTh[:, qs0:qs0 + ncols],
                                        start=True, stop=True)
                                    P = pC.tile([128, 512], f32r,
                                                tag="P", bufs=3)
                                    nc.scalar.activation(
                                        P[:, 0:ncols], sc[:, 0:ncols],
                                        AF.Exp, scale=SCALE)
                                    if j >= 0:
                                        nc.vector.tensor_mul(
                                            P[:, 0:128], P[:, 0:128],
                                            tri_t[:])
                                    nc.tensor.matmul(
                                        acc[:, coloff:512],
                                        v_nat[:, kt, :], P[:, 0:ncols],
                                        start=(kt == 0),
                                        stop=(kt == nkt - 1))
                                    nc.tensor.matmul(
                                        den[0:1, coloff:512], ones_t[:],
                                        P[:, 0:ncols],
                                        start=(kt == 0),
                                        stop=(kt == nkt - 1))
                                rd = pC.tile([1, 512], f32, tag="rd")
                                nc.vector.reciprocal(rd[:], den[:])
                                rb = pC.tile([128, 512], f32, tag="rb2")
                                nc.gpsimd.partition_broadcast(rb[:], rd[:])
                                ot = pC.tile([128, 512], f16, tag="ot")
                                nc.vector.tensor_mul(ot[:], acc[:], rb[:])
                                nc.sync.dma_start(
                                    _sb(outd[h, :, :], qb), ot[:])

                    # ---- Phase D: Wo partial + chunked AllReduce ----
                    with tc.tile_pool(name="pD", bufs=2) as pD, \
                         tc.tile_pool(name="pDw", bufs=1) as pDw, \
                         tc.tile_pool(name="pDp", bufs=2,
                                      space="PSUM") as pDp:
                        wo_t = pDw.tile([128, QH, NHT, 128], f16)
                        for k2 in range(QH):
                            nc.sync.dma_start(
                                wo_t[:, k2, :, :].rearrange(
                                    "p a b -> p (a b)"),
                                wof[k2 * 128:(k2 + 1) * 128, :])
                        for sb in range(NSB):
                            osb = pD.tile([128, QH, 512], f16,
                                          tag="osb", bufs=1)
                            nc.sync.dma_start(
                                osb[:],
                                outd[:, :, sb * 512:(sb + 1) * 512]
                                .rearrange("o p n -> p o n"))
                            for ocg in range(2):
                                xqg = pD.tile([128, 8, 512], f16,
                                              tag="xqg", bufs=1)
                                nc.sync.dma_start(
                                    xqg[:],
                                    xg[:].rearrange("(a p) n -> p a n",
                                                    p=128)
                                    [:, ocg * 8:(ocg + 1) * 8,
                                     sb * 512:(sb + 1) * 512])
                                for oc8 in range(8):
                                    oc = ocg * 8 + oc8
                                    ps = pDp.tile([128, 512], f32,
                                                  tag="y")
                                    for k2 in range(QH):
                                        nc.tensor.matmul(
                                            ps[:],
                                            wo_t[:, k2, oc, :],
                                            osb[:, k2, :],
                                            start=(k2 == 0),
                                            stop=(k2 == QH - 1))
                                    yt = pD.tile([128, 512], f32,
                                                 tag="yt")
                                    nc.vector.scalar_tensor_tensor(
                                        out=yt[:], in0=xqg[:, oc8, :],
                                        scalar=0.25, in1=ps[:],
                                        op0=AL.mult, op1=AL.add)
                                    nc.sync.dma_start(
                                        ar_in[sb][oc * 128:
                                                  (oc + 1) * 128, :],
                                        yt[:])
                            nc.gpsimd.collective_compute(
                                "AllReduce", AL.add,
                                replica_groups=GROUPS,
                                ins=[ar_in[sb].opt()],
                                outs=[ar_out[sb].opt()])

            # ---------- Phase E: x1 = xT + ar; rmsnorm2 -> h2T ----------
            with tc.tile_pool(name="ph2", bufs=1) as ph2:
                h2T = ph2.tile([128, NHT, S], f16)
                with tc.tile_pool(name="pE", bufs=1) as pE, \
                     tc.tile_pool(name="pEs", bufs=2) as pEs, \
                     tc.tile_pool(name="pEp", bufs=2, space="PSUM") as pEp:
                    for sb in range(NSB):
                        x1sb = pE.tile([128, NHT, 512], f32, tag="x1sb")
                        ss_ps = pEp.tile([1, 512], f32, tag="ss2")
                        for ht in range(NHT):
                            nc.sync.dma_start(
                                x1sb[:, ht, :],
                                ar_out[sb][ht * 128:(ht + 1) * 128, :])
                            sq = pEs.tile([128, 512], f32r, tag="sq2")
                            nc.scalar.activation(sq[:], x1sb[:, ht, :],
                                                 AF.Square)
                            nc.tensor.matmul(ss_ps[:], ones_t[:], sq[:],
                                             start=(ht == 0),
                                             stop=(ht == NHT - 1))
                        sd = pEs.tile([1, 512], f32, tag="sd2")
                        nc.scalar.activation(sd[:], ss_ps[:], AF.Sqrt,
                                             bias=eps_t[0:1, :],
                                             scale=1.0 / H)
                        rr = pEs.tile([1, 512], f32, tag="rr2")
                        nc.vector.reciprocal(rr[:], sd[:])
                        rb = pEs.tile([128, 512], f32, tag="rb3")
                        nc.gpsimd.partition_broadcast(rb[:], rr[:])
                        for ht in range(NHT):
                            nc.vector.scalar_tensor_tensor(
                                out=_sb(h2T[:, ht, :], sb),
                                in0=x1sb[:, ht, :],
                                scalar=wn2_t[:, ht:ht + 1],
                                in1=rb[:], op0=AL.mult, op1=AL.mult)

                # ---------- Phase F1: gate/up/silu-mul -> mT (DRAM) -------
                with tc.tile_pool(name="pF", bufs=2) as pF, \
                     tc.tile_pool(name="pFw", bufs=2) as pFw, \
                     tc.tile_pool(name="pFp", bufs=2, space="PSUM") as pFp:
                    for ct in range(FCT):
                        wgt = pFw.tile([128, NHT, 128], f16, tag="wg")
                        wut = pFw.tile([128, NHT, 128], f16, tag="wu")
                        nc.sync.dma_start(
                            wgt[:],
                            wgf[:].rearrange("(o p) n -> p o n", p=128)
                               [:, :, ct * 128:(ct + 1) * 128])
                        nc.sync.dma_start(
                            wut[:],
                            wuf[:].rearrange("(o p) n -> p o n", p=128)
                               [:, :, ct * 128:(ct + 1) * 128])
                        for sb in range(NSB):
                            pg = pFp.tile([128, 512], f32, tag="pg")
                            pu = pFp.tile([128, 512], f32, tag="pu")
                            for ht in range(NHT):
                                nc.tensor.matmul(
                                    pg[:], wgt[:, ht, :],
                                    _sb(h2T[:, ht, :], sb),
                                    start=(ht == 0), stop=(ht == NHT - 1))
                            for ht in range(NHT):
                                nc.tensor.matmul(
                                    pu[:], wut[:, ht, :],
                                    _sb(h2T[:, ht, :], sb),
                                    start=(ht == 0), stop=(ht == NHT - 1))
                            sg = pF.tile([128, 512], f32, tag="sg")
                            nc.scalar.activation(sg[:], pg[:], AF.Silu)
                            mt = pF.tile([128, 512], f16, tag="mt")
                            nc.vector.tensor_mul(mt[:], pu[:], sg[:])
                            nc.sync.dma_start(
                                _sb(mTd[ct, :, :], sb), mt[:])

            # ---------- Phase F2: down + 0.25*x1 -> chunked RS --------
            with tc.tile_pool(name="pwd", bufs=1) as pwd, \
                 tc.tile_pool(name="pGm", bufs=1) as pGm, \
                 tc.tile_pool(name="pG", bufs=2) as pG, \
                 tc.tile_pool(name="pGp", bufs=2, space="PSUM") as pGp:
                mm = pGm.tile([128, FCT, S], f16)
                for ct in range(FCT):
                    nc.sync.dma_start(mm[:, ct, :], mTd[ct, :, :])
                for oc in range(NHT):
                    wdo = pwd.tile([128, FCT, 128], f16, tag="wdo",
                                   bufs=2)
                    nc.sync.dma_start(
                        wdo[:],
                        wdf[:].rearrange("(a p) n -> p a n", p=128)
                        [:, :, oc * 128:(oc + 1) * 128])
                    for sb in range(NSB):
                        ps = pGp.tile([128, 512], f32, tag="pd")
                        for ct in range(FCT):
                            nc.tensor.matmul(
                                ps[:], wdo[:, ct, :],
                                mm[:, ct, sb * 512:(sb + 1) * 512],
                                start=(ct == 0), stop=(ct == FCT - 1))
                        x1t = pG.tile([128, 512], f32, tag="x1t")
                        nc.sync.dma_start(
                            x1t[:],
                            ar_out[sb][oc * 128:(oc + 1) * 128, :])
                        yd = pG.tile([128, 512], f16, tag="yd")
                        nc.vector.scalar_tensor_tensor(
                            out=yd[:], in0=x1t[:], scalar=0.25,
                            in1=ps[:], op0=AL.mult, op1=AL.add)
                        nc.sync.dma_start(
                            rs_in[oc // 8, sb,
                                  (oc % 8) * 128:(oc % 8 + 1) * 128, :],
                            yd[:])
                    if oc % 8 == 7:
                        hh = oc // 8
                        nc.gpsimd.collective_compute(
                            "ReduceScatter", AL.add, replica_groups=GROUPS,
                            ins=[rs_in[hh].opt()],
                            outs=[rs_out[hh * 1024:(hh + 1) * 1024, :]
                                  .opt()])

            # ---------- Phase G: write output ----------
            nc.sync.dma_start(outsl[:], rs_out[:])

    nc.finalize()
    return nc


_CACHE = {}


def _get_nc():
    if "nc" not in _CACHE:
        _CACHE["nc"] = build()
    return _CACHE["nc"]


# Inputs that depend only on the weights/rope/norm tensors (not on x).
# These stay device-resident across calls; a full content-equality check
# on the raw inputs guards correctness for arbitrary inputs.
_STATIC_NAMES = ("wqh", "wkh", "wvh", "woh", "wgh", "wuh", "wdh",
                 "cosT", "sinT", "wn1", "wn2", "tri", "ones", "epsb",
                 "ident")
_STATIC_RAW_KEYS = ("Wq", "Wk", "Wv", "Wo", "Wgate", "Wup", "Wdown",
                    "w_norm1", "w_norm2", "freqs_cos", "freqs_sin")


def _get_runner():
    """Build the jitted shard_map executable ONCE; reuse across calls."""
    if "runner" in _CACHE:
        return _CACHE["runner"]

    import jax
    import jax.numpy as jnp
    from jax.sharding import Mesh, PartitionSpec, NamedSharding
    from jax.experimental.shard_map import shard_map
    from concourse import bass2jax

    bass2jax.install_neuronx_cc_hook()
    nc = _get_nc()
    n_cores = 8

    partition_name = (nc.partition_id_tensor.name
                      if nc.partition_id_tensor else None)
    in_names, out_names, out_avals, zero_outs = [], [], [], []
    for alloc in nc.m.functions[0].allocations:
        if not isinstance(alloc, mybir.MemoryLocationSet):
            continue
        name = alloc.memorylocations[0].name
        if alloc.kind == "ExternalInput":
            if name != partition_name:
                in_names.append(name)
        elif alloc.kind == "ExternalOutput":
            shape = tuple(alloc.tensor_shape)
            dtype = mybir.dt.np(alloc.dtype)
            out_names.append(name)
            out_avals.append(jax.core.ShapedArray(shape, dtype))
            zero_outs.append(np.zeros(shape, dtype))
    n_params = len(in_names)
    n_outs = len(out_avals)
    all_names = list(in_names) + list(out_names)
    if partition_name is not None:
        all_names.append(partition_name)
    donate = tuple(range(n_params, n_params + n_outs))

    def _body(*args):
        operands = list(args)
        if partition_name is not None:
            operands.append(bass2jax.partition_id_tensor())
        outs = bass2jax._bass_exec_p.bind(
            *operands,
            out_avals=tuple(out_avals),
            in_names=tuple(all_names),
            out_names=tuple(out_names),
            lowering_input_output_aliases=(),
            sim_require_finite=True,
            sim_require_nnan=True,
            nc=nc,
        )
        return tuple(outs)

    devices = jax.devices()[:n_cores]
    mesh = Mesh(np.asarray(devices), ("core",))
    sh = NamedSharding(mesh, PartitionSpec("core"))
    in_specs = (PartitionSpec("core"),) * (n_params + n_outs)
    out_specs = (PartitionSpec("core"),) * n_outs
    sharded = jax.jit(
        shard_map(_body, mesh=mesh, in_specs=in_specs,
                  out_specs=out_specs, check_rep=False),
        donate_argnums=donate,
        keep_unused=True,
    )

    # donated zero output buffers, generated on device (no host upload)
    zero_shapes = [(n_cores * z.shape[0], *z.shape[1:]) for z in zero_outs]
    zero_dtypes = [z.dtype for z in zero_outs]
    make_zeros = jax.jit(
        lambda: tuple(jnp.zeros(s, d)
                      for s, d in zip(zero_shapes, zero_dtypes)),
        out_shardings=tuple(sh for _ in zero_shapes),
    )

    import os
    from concurrent.futures import ThreadPoolExecutor
    dbg = bool(os.environ.get("KERNEL_DEBUG_TIMING"))
    pool = ThreadPoolExecutor(max_workers=n_cores)

    def _zeros():
        z = _CACHE.pop("next_zeros", None)
        return z if z is not None else make_zeros()

    def run(x_glob, static_dev, assemble):
        """x_glob: device (or host) global xs array; static_dev: dict of
        device-resident static inputs; assemble(c, shard) consumes the
        per-core output shard as it arrives."""
        import time as _time
        t0 = _time.time()
        args = []
        for name in in_names:
            if name == "xs":
                args.append(x_glob)
            else:
                args.append(static_dev[name])
        out_arrs = sharded(*args, *_zeros())
        t1 = _time.time()
        # pre-create the next call's donated zero buffers while the
        # kernel executes
        _CACHE["next_zeros"] = make_zeros()
        out = out_arrs[0]
        shards = sorted(out.addressable_shards,
                        key=lambda s: s.device.id)
        datas = list(pool.map(lambda s: np.asarray(s.data), shards))
        t2 = _time.time()
        for c, d in enumerate(datas):
            assemble(c, d.reshape(out_avals[0].shape))
        t3 = _time.time()
        if dbg:
            print(f"[run] dispatch: {t1-t0:.3f}s  exec+fetch: "
                  f"{t2-t1:.3f}s  assemble: {t3-t2:.3f}s", flush=True)

    def put_x(shard_fn):
        """Prep + async-upload the 8 per-core x shards, interleaved so
        the transfer of shard c overlaps the prep of shard c+1."""
        arrs = []
        for c in range(n_cores):
            arrs.append(jax.device_put(shard_fn(c), devices[c]))
        return jax.make_array_from_single_device_arrays(
            (n_cores * 512, S), sh, arrs)

    def put_static(in_maps):
        """Upload the static inputs once; returns dict of device arrays."""
        dev = {}
        for name in _STATIC_NAMES:
            glob = np.concatenate(
                [np.asarray(in_maps[c][name]) for c in range(n_cores)],
                axis=0)
            dev[name] = jax.device_put(glob, sh)
        for a in dev.values():
            a.block_until_ready()
        return dev

    _CACHE["runner"] = (run, put_static, put_x)
    return _CACHE["runner"]


def _host_prep_static(inputs):
    """Per-core maps for the weight-derived (x-independent) inputs."""
    Wq = np.asarray(inputs["Wq"], np.float32).astype(np.float16)
    Wk = np.asarray(inputs["Wk"], np.float32).astype(np.float16)
    Wv = np.asarray(inputs["Wv"], np.float32).astype(np.float16)
    Wo = np.asarray(inputs["Wo"], np.float32).astype(np.float16)
    Wg = np.asarray(inputs["Wgate"], np.float32).astype(np.float16)
    Wu = np.asarray(inputs["Wup"], np.float32).astype(np.float16)
    Wd = np.asarray(inputs["Wdown"], np.float32).astype(np.float16)
    wn1v = np.asarray(inputs["w_norm1"], np.float32)
    wn2v = np.asarray(inputs["w_norm2"], np.float32)
    cos = np.asarray(inputs["freqs_cos"], np.float32)
    sin = np.asarray(inputs["freqs_sin"], np.float32)

    tri_np = (np.arange(128)[None, :] >= np.arange(128)[:, None])
    tri_np = tri_np.astype(np.float32)
    wn1_np = np.ascontiguousarray(wn1v.reshape(NHT, 128).T)
    wn2_np = np.ascontiguousarray(wn2v.reshape(NHT, 128).T)

    shared = dict(cosT=np.ascontiguousarray(cos.T),
                  sinT=np.ascontiguousarray(sin.T),
                  wn1=wn1_np, wn2=wn2_np, tri=tri_np,
                  ones=np.ones((128, 1), np.float32),
                  ident=np.eye(128, dtype=np.float32),
                  epsb=np.full((128, 1), EPS, np.float32))

    halves = []      # halves[dp][tp] -> dict of weight-half arrays
    for dp in range(2):
        r0, r1 = dp * 1024, (dp + 1) * 1024
        per_tp = []
        for tp in range(TPN):
            qcols = []
            for h in range(tp * QH, (tp + 1) * QH):
                qcols.extend(h * HD + PERM)
            per_tp.append(dict(
                wqh=Wq[r0:r1][:, qcols],
                wkh=Wk[r0:r1][:, tp * HD + PERM],
                wvh=np.ascontiguousarray(
                    Wv[r0:r1, tp * HD:(tp + 1) * HD]),
                woh=np.ascontiguousarray(
                    Wo[tp * QH * HD + dp * 256:
                       tp * QH * HD + (dp + 1) * 256, :]),
                wgh=np.ascontiguousarray(
                    Wg[r0:r1, tp * FFS:(tp + 1) * FFS]),
                wuh=np.ascontiguousarray(
                    Wu[r0:r1, tp * FFS:(tp + 1) * FFS]),
                wdh=np.ascontiguousarray(
                    Wd[tp * FFS + dp * 704:tp * FFS + (dp + 1) * 704, :]),
            ))
        halves.append(per_tp)

    in_maps = []
    for c in range(8):
        dp, tp = c // 4, c % 4
        m = dict(shared)
        m.update(halves[dp][tp])
        in_maps.append(m)
    return in_maps


def _prep_x_shard(x, c):
    dp, tp = c // 4, c % 4
    buf = np.empty((512, S), np.float16)
    buf[:] = x[dp][:, tp * 512:(tp + 1) * 512].T
    return buf


def _statics_unchanged(inputs):
    cached = _CACHE.get("static_raw")
    if cached is None:
        return False
    for k in _STATIC_RAW_KEYS:
        a, b = cached[k], inputs[k]
        if a is b:
            continue
        a = np.asarray(a)
        b = np.asarray(b)
        if a.shape != b.shape or a.dtype != b.dtype or \
                not np.array_equal(a, b):
            return False
    return True


def kernel(**inputs) -> np.ndarray:
    run, put_static, put_x = _get_runner()
    if not _statics_unchanged(inputs):
        smaps = _host_prep_static(inputs)
        _CACHE["static_dev"] = put_static(smaps)
        _CACHE["static_raw"] = {k: inputs[k] for k in _STATIC_RAW_KEYS}
    x = np.asarray(inputs["x"], np.float32)
    x_glob = put_x(lambda c: _prep_x_shard(x, c))
    out = np.empty((B, S, H), np.float32)

    def assemble(c, sl):                      # sl: [H, 512] fp16
        dp, tp = c // 4, c % 4
        out[dp, tp * SSL:(tp + 1) * SSL, :] = sl.T

    run(x_glob, _CACHE["static_dev"], assemble)
    return out
